# revision 24
# baseline (speedup 1.0000x reference)
"""Biased self-attention TRN2 Bass kernel (8 NeuronCores), v6.

Problem: nn_BiasedSelfAttention — B=2, N=2048, D=1024, H=16, DK=64.
    q,k,v = split_heads(x@Wq+bq), ...; k,v scaled by (1+alpha[b,n]);
    logits = q k^T/sqrt(DK) + bias[b][None]; y = softmax(logits) v;
    out = merge_heads(y) @ Wo + bo.

Sharding: 8 cores = (batch b in {0,1}) x (head-group hg in {0..3} of 4
heads = 256 dims of D).  Data parallel over B, tensor parallel over H.
Each core computes a partial O-projection (its 256 rows of Wo); the
host sums the 4 partials per batch and adds bo.

v6 design (v5 profile: rounds fully ACT-bound at 1325ns because the
FD=512 exp pays ~260ns/instr overhead twice; tail_b at round 4 of each
quarter blocked the PE FIFO ~7us on the reciprocal's SBUF->SBUF DMA
chain; PSUM had no spare banks to decouple projections from QK):
  - quarters split into two 16-round PASSES (head pair = pass).  PSUM:
    qk s-tag [128,2,512] x2 (4 banks) + y [65,2,512] (2 banks) + small
    s-tag x2 (2 banks) = 8.  Projections/O-proj/tail use the small tag
    so they never steal the QK double-buffer.
  - exp is ONE FD=1024 ACTIVATE per round again (PSUM source, per-key
    (1+alpha) scale AP); DVE bf16 2x multiply by host-precomputed
    exp(bias); ebias tiles are loaded once per quarter and reused by
    both passes.
  - denominators: per-pass DVE reciprocal directly on the single-
    partition accumulator row -- the DMA reshape chain is gone.
  - k/q weights are loaded in hp halves so round 0 starts ~21us in.
"""

import json
import sys

sys.path.insert(0, "/opt/trn_rl_repo")

import numpy as np
import ml_dtypes

import concourse.bass as bass
import concourse.mybir as mybir
import concourse.tile as tile
from concourse.bass_utils import run_bass_kernel_spmd

# ---------------------------------------------------------------- bir fix --
# The pinned walrus encodes at most ONE sem-wait per instruction, but Tile's
# wait-assigner can emit several.  Hoist extras onto EventSemaphore
# instructions just before the instruction.


def _split_multi_waits(bir_json: bytes) -> bytes:
    m = json.loads(bir_json)
    for fn in m.get("functions", []):
        for blk in fn.get("blocks", []):
            insts = blk.get("instructions")
            if not insts:
                continue
            out = []
            for inst in insts:
                sync = inst.get("sync_info")
                waits = (sync or {}).get("on_wait") or []
                if len(waits) > 1:
                    for i, w in enumerate(waits[:-1]):
                        out.append({
                            "debug": inst.get("debug", 0),
                            "engine": inst["engine"],
                            "ins": [],
                            "name": f"{inst['name']}-sw{i}",
                            "opcode": "EventSemaphore",
                            "outs": [],
                            "sync_info": {"on_update": [], "on_wait": [w]},
                        })
                    sync["on_wait"] = waits[-1:]
                out.append(inst)
            blk["instructions"] = out
    return json.dumps(m).encode()


def _patch_bass():
    if getattr(bass.Bass, "_multiwait_patched", False):
        return
    orig = bass.Bass.to_json_bytes

    def to_json_bytes(self, *a, **kw):
        return _split_multi_waits(orig(self, *a, **kw))

    bass.Bass.to_json_bytes = to_json_bytes
    bass.Bass._multiwait_patched = True


_patch_bass()


def _patch_ldw_opt():
    """Enable walrus's redundant-LDWEIGHTS elimination (off by default in
    bass_utils).  Consecutive matmuls that share a stationary operand then
    load it once."""
    import concourse.bass_utils as _bu
    if getattr(_bu, "_ldw_opt_patched", False):
        return
    orig = _bu.run_command

    def run_command(cmd, *a, **kw):
        # ldw-opt=true crashes walrus codegen (visitInstLdweights) on this
        # pinned compiler -- keep the flag off; wrapper retained as a hook.
        return orig(cmd, *a, **kw)

    _bu.run_command = run_command
    _bu._ldw_opt_patched = True


_patch_ldw_opt()

# ------------------------------------------------------------- dimensions --
B, N, D, H = 2, 2048, 1024, 16
DK = D // H                      # 64
NCORES = 8
HPC = H // 4                     # 4 heads per core
DSL = HPC * DK                   # 256 D-columns per core
NQ4 = N // 512                   # 4 query quarters
MT = N // 128                    # 16 key tiles
F32 = mybir.dt.float32
F32R = mybir.dt.float32r
BF16 = mybir.dt.bfloat16
Exp = mybir.ActivationFunctionType.Exp
Copy = mybir.ActivationFunctionType.Copy
Add = mybir.AluOpType.add
Mult = mybir.AluOpType.mult


def _build_nc() -> bass.Bass:
    nc = bass.Bass()

    xT = nc.dram_tensor("xT", [128, 4, 8, 512], BF16, kind="ExternalInput")
    wq2 = nc.dram_tensor("wq2", [128, 2, 8, 128], BF16, kind="ExternalInput")
    wk2 = nc.dram_tensor("wk2", [128, 2, 8, 128], BF16, kind="ExternalInput")
    wv = nc.dram_tensor("wv", [128, 8, DSL], BF16, kind="ExternalInput")
    wo = nc.dram_tensor("wo", [128, 2, D], BF16, kind="ExternalInput")
    ebiasT = nc.dram_tensor("ebiasT", [N, N], BF16, kind="ExternalInput")
    bv_r = nc.dram_tensor("bv_r", [1, DSL], BF16, kind="ExternalInput")
    bq_col = nc.dram_tensor("bq_col", [128, 2], F32, kind="ExternalInput")
    bk_col = nc.dram_tensor("bk_col", [128, 2], F32, kind="ExternalInput")
    scol = nc.dram_tensor("scol", [128, MT], F32, kind="ExternalInput")
    ones64 = nc.dram_tensor("ones64", [65, 64], F32R, kind="ExternalInput")
    onescol = nc.dram_tensor("onescol", [128, 1], BF16, kind="ExternalInput")
    onesr = nc.dram_tensor("onesr", [1, 128], BF16, kind="ExternalInput")
    identb = nc.dram_tensor("identb", [128, 128], BF16, kind="ExternalInput")
    out_part = nc.dram_tensor("out_part", [N, D], BF16, kind="ExternalOutput")

    with tile.TileContext(nc) as tc:
        with tc.tile_pool(name="consts", bufs=1) as consts, \
             tc.tile_pool(name="persist", bufs=1) as persist, \
             tc.tile_pool(name="stream", bufs=4) as stream, \
             tc.tile_pool(name="work", bufs=3) as work, \
             tc.tile_pool(name="outp", bufs=2) as outp, \
             tc.tile_pool(name="psum", bufs=1, space="PSUM") as pp:

            # ---- constants -------------------------------------------------
            xT_sb = consts.tile([128, 4, 8, 512], BF16, tag="xT")
            wq_t = consts.tile([128, 2, 8, 128], BF16, tag="wq")
            wk_t = consts.tile([128, 2, 8, 128], BF16, tag="wk")
            wv_t = consts.tile([128, 8, DSL], BF16, tag="wv")
            wo_t = consts.tile([128, 2, D], BF16, tag="wo")
            identb_t = consts.tile([128, 128], BF16, tag="identb")
            bv_t = consts.tile([1, DSL], BF16, tag="bv")
            bq_c = consts.tile([128, 2], F32, tag="bqc")
            bk_c = consts.tile([128, 2], F32, tag="bkc")
            scol_t = consts.tile([128, MT], F32, tag="scol")
            ones64_t = consts.tile([65, 64], F32R, tag="ones64")
            onescol_t = consts.tile([128, 1], BF16, tag="onescol")
            onesr_t = consts.tile([1, 128], BF16, tag="onesr")
            # DMA order = arrival order (~175 GB/s effective, ~9us startup).
            nc.sync.dma_start(out=identb_t, in_=identb[:])
            nc.sync.dma_start(out=onescol_t, in_=onescol[:])
            nc.sync.dma_start(out=onesr_t, in_=onesr[:])
            nc.sync.dma_start(out=bq_c, in_=bq_col[:])
            nc.sync.dma_start(out=bk_c, in_=bk_col[:])
            nc.sync.dma_start(out=scol_t, in_=scol[:])
            nc.sync.dma_start(out=xT_sb[:, 0], in_=xT[:, 0])
            nc.sync.dma_start(out=wk_t[:, 0], in_=wk2[:, 0])
            nc.sync.dma_start(out=wq_t[:, 0], in_=wq2[:, 0])
            nc.sync.dma_start(out=wv_t, in_=wv[:])
            nc.sync.dma_start(out=bv_t, in_=bv_r[:])
            nc.sync.dma_start(out=ones64_t, in_=ones64[:])

            # ---- persistent intermediates ---------------------------------
            # q^T/k^T: [dk-pair row hi*64+dk, head-pair hp, n]; kT UNSCALED
            qT_all = persist.tile([128, 2, N], BF16, tag="qT")
            kT_all = persist.tile([128, 2, N], BF16, tag="kT")
            # v (scaled) + ones col: [m-part, m-tile, head, 65]
            vaug = persist.tile([128, MT, HPC, 65], BF16, tag="vaug")
            # normalized y^T for O-proj
            yT_all = persist.tile([128, 2, N], BF16, tag="yT")
            # per-quarter y + denominators staging
            y_sb = persist.tile([65, HPC, 512], F32R, tag="ysb")

            # vaug ones columns, written once
            nc.vector.tensor_copy(
                vaug[:, :, :, 64:65],
                onescol_t.unsqueeze(1).unsqueeze(1)
                .broadcast_to([128, MT, HPC, 1]))

            state = {}

            def eb_load(q4, mt):
                eb_t = stream.tile([128, 512], BF16, tag="ebias", bufs=18,
                                   name=f"b{q4}_{mt}")
                nc.sync.dma_start(
                    out=eb_t,
                    in_=ebiasT[mt * 128:mt * 128 + 128,
                               q4 * 512:q4 * 512 + 512])
                state[("eb", q4, mt)] = eb_t

            # prefetch ALL q0 bias tiles, interleaved with the remaining
            # x blocks in exact deadline order: kproj_h(1,0) at round ~2
            # needs xT1 almost immediately, so only two bias tiles go
            # ahead of it; later blocks have progressively more slack.
            nc.sync.dma_start(out=xT_sb[:, 1], in_=xT[:, 1])
            eb_load(0, 0)
            eb_load(0, 1)
            eb_load(0, 2)
            eb_load(0, 3)
            nc.sync.dma_start(out=xT_sb[:, 2], in_=xT[:, 2])
            for mt in range(4, 10):
                eb_load(0, mt)
            nc.sync.dma_start(out=xT_sb[:, 3], in_=xT[:, 3])
            for mt in range(10, 16):
                eb_load(0, mt)
            nc.sync.dma_start(out=wk_t[:, 1], in_=wk2[:, 1])
            nc.sync.dma_start(out=wq_t[:, 1], in_=wq2[:, 1])
            nc.sync.dma_start(out=wo_t, in_=wo[:])

            # ---- projections (small s-tag PSUM, interleaved into rounds) --
            def kproj_h(c, hp):
                nsl = slice(c * 512, c * 512 + 512)
                ps = pp.tile([128, 512], F32, tag="s", bufs=2,
                             name=f"kps{c}_{hp}")
                for t in range(8):
                    nc.tensor.matmul(
                        ps, wk_t[:, hp, t, :], xT_sb[:, c, t, :],
                        start=(t == 0), stop=(t == 7))
                nc.vector.tensor_scalar(
                    kT_all[:, hp, nsl], ps, bk_c[:, hp:hp + 1], None, Add)

            def qproj_h(c, hp):
                nsl = slice(c * 512, c * 512 + 512)
                ps = pp.tile([128, 512], F32, tag="s", bufs=2,
                             name=f"qps{c}_{hp}")
                for t in range(8):
                    nc.tensor.matmul(
                        ps, wq_t[:, hp, t, :], xT_sb[:, c, t, :],
                        start=(t == 0), stop=(t == 7))
                nc.vector.tensor_scalar(
                    qT_all[:, hp, nsl], ps,
                    0.125, bq_c[:, hp:hp + 1], Mult, Add)

            def vproj(mt):
                mb, mo = divmod(mt, 4)
                vp = pp.tile([128, 256], F32, tag="s", bufs=2,
                             name=f"vps{mt}")
                for t in range(8):
                    nc.tensor.matmul(
                        vp, xT_sb[:, mb, t, mo * 128:mo * 128 + 128],
                        wv_t[:, t, :], start=(t == 0), stop=False)
                nc.tensor.matmul(
                    vp, onesr_t[0:1, :], bv_t[0:1, :], start=False, stop=True)
                vr = vp.rearrange("p (h d) -> p h d", h=HPC)
                nc.vector.tensor_scalar(
                    vaug[:, mt, :, 0:64], vr,
                    scol_t[:, mt:mt + 1], None, Mult)

            warm = pp.tile([128, 512], F32, tag="s", bufs=2, name="warm")
            for w in range(52):
                nc.tensor.matmul(warm[:, 0:128], identb_t, identb_t,
                                 start=(w == 0), stop=(w == 51))

            # minimum to start round 0 of pass (0,0): kT/qT chunk 0 only.
            # vproj(0..) rides the round inserts -- AV can lag a few rounds
            # behind the exp cadence (a/e bufs below give the headroom).
            kproj_h(0, 0)
            qproj_h(0, 0)

            # ---- round bodies ---------------------------------------------
            def qk_round(q4, p, mt):
                nsl = slice(q4 * 512, q4 * 512 + 512)
                if p == 0:
                    if ("eb", q4, mt) not in state:
                        eb_load(q4, mt)
                    # keep a 2-round DMA lookahead so the multiply never
                    # waits on a same-round bias-tile transfer
                    if mt + 2 < MT and ("eb", q4, mt + 2) not in state:
                        eb_load(q4, mt + 2)
                eb_t = state[("eb", q4, mt)]
                if p == 1:
                    del state[("eb", q4, mt)]
                s_ps = pp.tile([128, 2, 512], F32, tag="s2", bufs=2,
                               name=f"s{q4}_{p}_{mt}")
                for hi in range(2):
                    nc.tensor.matmul(
                        s_ps[:, hi],
                        kT_all[hi * 64:hi * 64 + 64, p,
                               mt * 128:mt * 128 + 128],
                        qT_all[hi * 64:hi * 64 + 64, p, nsl],
                        start=True, stop=True)
                e_t = work.tile([128, 2, 512], BF16, tag="e", bufs=6,
                                name=f"e{q4}_{p}_{mt}")
                # per-key-partition (1+alpha) scale rides the exp
                nc.scalar.activation(e_t, s_ps, Exp,
                                     scale=scol_t[:, mt:mt + 1])
                a_t = work.tile([128, 2, 512], BF16, tag="a", bufs=6,
                                name=f"a{q4}_{p}_{mt}")
                nc.vector.tensor_mul(
                    a_t, e_t,
                    eb_t.unsqueeze(1).broadcast_to([128, 2, 512]))
                state[("a", mt % 6)] = a_t

            def av_round(q4, p, mt):
                a_t = state[("a", mt % 6)]
                y_ps = state["y"]
                for hi in range(2):
                    nc.tensor.matmul(
                        y_ps[:, hi], vaug[:, mt, 2 * p + hi, :], a_t[:, hi],
                        start=(mt == 0), stop=(mt == MT - 1))

            def tail_b(q4, hq):
                # one head-pair: 2 recip-broadcast matmuls (PE) + 2 muls (DVE)
                r_row = state[("rrow", q4, hq)]
                for hi in range(2):
                    h = hq * 2 + hi
                    rb = pp.tile([128, 512], F32, tag="s", bufs=2,
                                 name=f"rb{q4}_{h}")
                    nc.tensor.matmul(
                        rb[0:64, :], ones64_t[0:1, :],
                        r_row[0:1, hi, :], start=True, stop=True)
                    nc.vector.tensor_mul(
                        yT_all[hi * 64:hi * 64 + 64, hq,
                               q4 * 512:q4 * 512 + 512],
                        y_sb[0:64, h, :].bitcast(F32), rb[0:64, :])

            def oproj_full(q4, j, tail=False):
                # both dc halves together: the yT stationary operand is
                # shared by consecutive matmuls, so with ldw-opt walrus
                # loads it once per hp instead of once per matmul.
                nt = q4 * 4 + j
                o_ps = [pp.tile([128, 512], F32, tag="s", bufs=2,
                                name=f"o{nt}_{dc}") for dc in range(2)]
                for hp in range(2):
                    for dc in range(2):
                        nc.tensor.matmul(
                            o_ps[dc],
                            yT_all[:, hp, nt * 128:nt * 128 + 128],
                            wo_t[:, hp, dc * 512:dc * 512 + 512],
                            start=(hp == 0), stop=(hp == 1))
                for dc in range(2):
                    o_sb = outp.tile([128, 512], BF16, tag="osb",
                                     name=f"ob{nt}_{dc}")
                    # in the final tail ACT is idle: alternate copy engines
                    if tail and dc == 1:
                        nc.scalar.copy(o_sb, o_ps[dc])
                    else:
                        nc.vector.tensor_copy(o_sb, o_ps[dc])
                    # gpsimd queue: keeps compute-gated stores from head-of-
                    # line blocking the ebias loads on the sync queue
                    (nc.sync if tail and dc == 1 else nc.gpsimd).dma_start(
                        out=out_part[nt * 128:nt * 128 + 128,
                                     dc * 512:dc * 512 + 512], in_=o_sb)

            # insertion schedules: {(q4==0, p): {mt: [fns]}} built inline
            def extra(q4, p, mt):
                if q4 == 0 and p == 0:
                    sched = {
                        1: [lambda: vproj(1)],
                        2: [lambda: kproj_h(1, 0), lambda: vproj(2)],
                        3: [lambda: vproj(3)], 4: [lambda: vproj(4)],
                        5: [lambda: vproj(5)],
                        6: [lambda: kproj_h(2, 0), lambda: vproj(6)],
                        7: [lambda: vproj(7)], 8: [lambda: vproj(8)],
                        9: [lambda: vproj(9)],
                        10: [lambda: kproj_h(3, 0), lambda: vproj(10)],
                        11: [lambda: vproj(11)], 12: [lambda: vproj(12)],
                        13: [lambda: vproj(13)],
                        14: [lambda: vproj(14), lambda: qproj_h(0, 1)],
                        15: [lambda: vproj(15), lambda: kproj_h(0, 1)],
                    }
                elif q4 == 0 and p == 1:
                    sched = {
                        2: [lambda: kproj_h(1, 1)],
                        4: [lambda: kproj_h(2, 1), lambda: qproj_h(1, 0)],
                        6: [lambda: kproj_h(3, 1), lambda: qproj_h(1, 1)],
                        10: [lambda: tail_b(0, 0)],
                        13: [lambda: eb_load(1, 0)],
                        14: [lambda: eb_load(1, 1)],
                    }
                elif p == 0:
                    sched = {8: [lambda: tail_b(q4 - 1, 1)]}
                    for k in range(4):
                        sched[9 + 2 * k] = [
                            (lambda jj: lambda: oproj_full(q4 - 1, jj))(k)]
                else:
                    sched = {
                        10: [lambda: tail_b(q4, 0)],
                    }
                    if q4 < NQ4 - 1:
                        sched[4] = [lambda: qproj_h(q4 + 1, 0)]
                        sched[6] = [lambda: qproj_h(q4 + 1, 1)]
                        sched[13] = [lambda: eb_load(q4 + 1, 0)]
                        sched[14] = [lambda: eb_load(q4 + 1, 1)]
                for fn in sched.get(mt, ()):
                    fn()

            # ---- main pass loop -------------------------------------------
            for q4 in range(NQ4):
                for p in range(2):
                    state["y"] = pp.tile([65, 2, 512], F32, tag="y", bufs=1,
                                         name=f"y{q4}_{p}")
                    qk_round(q4, p, 0)
                    if q4 == 0 and p == 0:
                        vproj(0)
                    for mt in range(1, MT):
                        qk_round(q4, p, mt)
                        av_round(q4, p, mt - 1)
                        extra(q4, p, mt)
                    av_round(q4, p, MT - 1)
                    y_ps = state.pop("y")
                    nc.vector.tensor_copy(
                        y_sb[:, 2 * p:2 * p + 2, :], y_ps)
                    # denominators for this head pair: direct reciprocal on
                    # the single-partition accumulator row (no DMA reshape)
                    # reshape the denom row onto 32 partitions (32 DMA
                    # descriptors), cheap 32-lane reciprocal, DMA back to a
                    # row.  Consumers (tail_b) are scheduled 10+ rounds out
                    # so the descriptor latency is hidden.
                    d_t = work.tile([32, 32], F32R, tag="dt", bufs=2,
                                    name=f"dt{q4}_{p}")
                    nc.sync.dma_start(
                        out=d_t, in_=y_sb[64:65, 2 * p:2 * p + 2, :])
                    d_r = work.tile([32, 32], F32R, tag="dr", bufs=2,
                                    name=f"dr{q4}_{p}")
                    nc.vector.reciprocal(out=d_r.bitcast(F32),
                                         in_=d_t.bitcast(F32))
                    r_row = work.tile([1, 2, 512], F32R, tag="rrow", bufs=2,
                                      name=f"rr{q4}_{p}")
                    nc.sync.dma_start(out=r_row, in_=d_r)
                    state[("rrow", q4, p)] = r_row

            # final quarter tail: bridge with keep-warm matmuls, then the
            # last normalize + O-proj.
            warm2 = pp.tile([128, 512], F32, tag="s", bufs=2, name="warm2")
            for w in range(24):
                nc.tensor.matmul(warm2, identb_t, kT_all[:, 0, 0:512],
                                 start=(w == 0), stop=(w == 23))
            tail_b(NQ4 - 1, 1)
            for j in range(4):
                oproj_full(NQ4 - 1, j, tail=True)

    return nc


def _ensure_ntff_hook():
    """Register the axon NTFF profiling hook if the agent image lacks
    antenv.axon_hooks (profiling only; kernel runs fine without)."""
    try:
        from antenv.axon_hooks import get_axon_ntff_profile_hook  # noqa: F401
        return
    except ImportError:
        pass
    import types
    import antenv
    from trn_agent_boot.trn_boot import _ntff_profile_via_ctypes

    mod = types.ModuleType("antenv.axon_hooks")
    holder = {}
    mod.set_axon_ntff_profile_hook = lambda h: holder.__setitem__("h", h)
    mod.get_axon_ntff_profile_hook = lambda: holder.get("h")
    sys.modules["antenv.axon_hooks"] = mod
    antenv.axon_hooks = mod
    mod.set_axon_ntff_profile_hook(
        _ntff_profile_via_ctypes("/opt/axon/libaxon_pjrt.so"))


_NC_CACHE: dict = {}


def _get_nc() -> bass.Bass:
    if "nc" not in _NC_CACHE:
        _NC_CACHE["nc"] = _build_nc()
    return _NC_CACHE["nc"]


def _col_layout(v):
    """[256] per-core head-slice -> [128, 2] f32: row (h%2)*64+dk, col h//2."""
    return np.ascontiguousarray(
        v.reshape(2, 2, 64).transpose(1, 2, 0).reshape(128, 2)
    ).astype(np.float32)


def _w_hp_layout(w, bf):
    """W[:, dsl] (1024, 256) -> [128, 2(hp), 8(t), 128] bf16."""
    a = w.astype(bf).reshape(8, 128, 2, 128)     # [t, part, hp, col]
    return np.ascontiguousarray(a.transpose(1, 2, 0, 3))


def kernel(x, alpha, bias, Wq, bq, Wk, bk, Wv, bv, Wo, bo, trace=False):
    bf = ml_dtypes.bfloat16
    x = np.asarray(x, np.float32)
    alpha = np.asarray(alpha, np.float32)
    bias = np.asarray(bias, np.float32)
    Wq = np.asarray(Wq, np.float32); bq = np.asarray(bq, np.float32)
    Wk = np.asarray(Wk, np.float32); bk = np.asarray(bk, np.float32)
    Wv = np.asarray(Wv, np.float32); bv = np.asarray(bv, np.float32)
    Wo = np.asarray(Wo, np.float32); bo = np.asarray(bo, np.float32)

    c = np.ascontiguousarray

    in_maps = []
    per_b = {}
    for b in range(B):
        s = 1.0 + alpha[b]                             # (N,)
        xt = x[b].T.astype(bf)                         # (D, N)
        per_b[b] = {
            # SBUF layout [p, block, t, 512] -> contiguous 8KB block rows
            "xT": c(xt.reshape(8, 128, 4, 512).transpose(1, 2, 0, 3)),
            "ebiasT": c(np.exp(bias[b].T).astype(bf)),  # (N, N) [m, n]
            # (1+alpha) laid out per key partition: [p, mt] = s[mt*128+p]
            "scol": c(s.reshape(MT, 128).T.astype(np.float32)),
        }
    for core in range(NCORES):
        b, hg = divmod(core, 4)
        dsl = slice(hg * DSL, hg * DSL + DSL)
        in_maps.append({
            **per_b[b],
            "wq2": _w_hp_layout(Wq[:, dsl], bf),
            "wk2": _w_hp_layout(Wk[:, dsl], bf),
            "wv": c(Wv[:, dsl].astype(bf).reshape(8, 128, DSL).transpose(1, 0, 2)),
            "wo": c(Wo[dsl, :].astype(bf).reshape(2, 128, D).transpose(1, 0, 2)),
            "bv_r": c(bv[dsl].reshape(1, DSL).astype(bf)),
            "bq_col": _col_layout(0.125 * bq[dsl]),
            "bk_col": _col_layout(bk[dsl]),
            "ones64": np.ones((65, 64), np.float32),
            "onescol": np.ones((128, 1), bf),
            "onesr": np.ones((1, 128), bf),
            "identb": np.eye(128, dtype=bf),
        })

    if trace:
        _ensure_ntff_hook()
    nc = _get_nc()
    res = run_bass_kernel_spmd(
        nc, in_maps, core_ids=list(range(NCORES)), trace=trace)

    out = np.zeros((B, N, D), np.float32)
    for core in range(NCORES):
        out[core // 4] += res.results[core]["out_part"].astype(np.float32)
    out += bo[None, None, :]
    if trace:
        kernel.last_exec_time_ns = res.exec_time_ns
        kernel.last_profile = res.profile_json
    return out


# revision 26
# speedup vs baseline: 1.0028x; 1.0028x over previous
"""Biased self-attention TRN2 Bass kernel (8 NeuronCores), v6.

Problem: nn_BiasedSelfAttention — B=2, N=2048, D=1024, H=16, DK=64.
    q,k,v = split_heads(x@Wq+bq), ...; k,v scaled by (1+alpha[b,n]);
    logits = q k^T/sqrt(DK) + bias[b][None]; y = softmax(logits) v;
    out = merge_heads(y) @ Wo + bo.

Sharding: 8 cores = (batch b in {0,1}) x (head-group hg in {0..3} of 4
heads = 256 dims of D).  Data parallel over B, tensor parallel over H.
Each core computes a partial O-projection (its 256 rows of Wo); the
host sums the 4 partials per batch and adds bo.

v6 design (v5 profile: rounds fully ACT-bound at 1325ns because the
FD=512 exp pays ~260ns/instr overhead twice; tail_b at round 4 of each
quarter blocked the PE FIFO ~7us on the reciprocal's SBUF->SBUF DMA
chain; PSUM had no spare banks to decouple projections from QK):
  - quarters split into two 16-round PASSES (head pair = pass).  PSUM:
    qk s-tag [128,2,512] x2 (4 banks) + y [65,2,512] (2 banks) + small
    s-tag x2 (2 banks) = 8.  Projections/O-proj/tail use the small tag
    so they never steal the QK double-buffer.
  - exp is ONE FD=1024 ACTIVATE per round again (PSUM source, per-key
    (1+alpha) scale AP); DVE bf16 2x multiply by host-precomputed
    exp(bias); ebias tiles are loaded once per quarter and reused by
    both passes.
  - denominators: per-pass DVE reciprocal directly on the single-
    partition accumulator row -- the DMA reshape chain is gone.
  - k/q weights are loaded in hp halves so round 0 starts ~21us in.
"""

import json
import sys

sys.path.insert(0, "/opt/trn_rl_repo")

import numpy as np
import ml_dtypes

import concourse.bass as bass
import concourse.mybir as mybir
import concourse.tile as tile
from concourse.bass_utils import run_bass_kernel_spmd

# ---------------------------------------------------------------- bir fix --
# The pinned walrus encodes at most ONE sem-wait per instruction, but Tile's
# wait-assigner can emit several.  Hoist extras onto EventSemaphore
# instructions just before the instruction.


def _split_multi_waits(bir_json: bytes) -> bytes:
    m = json.loads(bir_json)
    for fn in m.get("functions", []):
        for blk in fn.get("blocks", []):
            insts = blk.get("instructions")
            if not insts:
                continue
            out = []
            for inst in insts:
                sync = inst.get("sync_info")
                waits = (sync or {}).get("on_wait") or []
                if len(waits) > 1:
                    for i, w in enumerate(waits[:-1]):
                        out.append({
                            "debug": inst.get("debug", 0),
                            "engine": inst["engine"],
                            "ins": [],
                            "name": f"{inst['name']}-sw{i}",
                            "opcode": "EventSemaphore",
                            "outs": [],
                            "sync_info": {"on_update": [], "on_wait": [w]},
                        })
                    sync["on_wait"] = waits[-1:]
                out.append(inst)
            blk["instructions"] = out
    return json.dumps(m).encode()


def _patch_bass():
    if getattr(bass.Bass, "_multiwait_patched", False):
        return
    orig = bass.Bass.to_json_bytes

    def to_json_bytes(self, *a, **kw):
        return _split_multi_waits(orig(self, *a, **kw))

    bass.Bass.to_json_bytes = to_json_bytes
    bass.Bass._multiwait_patched = True


_patch_bass()


def _patch_ldw_opt():
    """Enable walrus's redundant-LDWEIGHTS elimination (off by default in
    bass_utils).  Consecutive matmuls that share a stationary operand then
    load it once."""
    import concourse.bass_utils as _bu
    if getattr(_bu, "_ldw_opt_patched", False):
        return
    orig = _bu.run_command

    def run_command(cmd, *a, **kw):
        # ldw-opt=true crashes walrus codegen (visitInstLdweights) on this
        # pinned compiler -- keep the flag off; wrapper retained as a hook.
        return orig(cmd, *a, **kw)

    _bu.run_command = run_command
    _bu._ldw_opt_patched = True


_patch_ldw_opt()

# ------------------------------------------------------------- dimensions --
B, N, D, H = 2, 2048, 1024, 16
DK = D // H                      # 64
NCORES = 8
HPC = H // 4                     # 4 heads per core
DSL = HPC * DK                   # 256 D-columns per core
NQ4 = N // 512                   # 4 query quarters
MT = N // 128                    # 16 key tiles
F32 = mybir.dt.float32
F32R = mybir.dt.float32r
BF16 = mybir.dt.bfloat16
Exp = mybir.ActivationFunctionType.Exp
Copy = mybir.ActivationFunctionType.Copy
Add = mybir.AluOpType.add
Mult = mybir.AluOpType.mult


def _build_nc() -> bass.Bass:
    nc = bass.Bass()

    xT = nc.dram_tensor("xT", [128, 4, 8, 512], BF16, kind="ExternalInput")
    wq2 = nc.dram_tensor("wq2", [128, 2, 8, 128], BF16, kind="ExternalInput")
    wk2 = nc.dram_tensor("wk2", [128, 2, 8, 128], BF16, kind="ExternalInput")
    wv = nc.dram_tensor("wv", [128, 8, DSL], BF16, kind="ExternalInput")
    wo = nc.dram_tensor("wo", [128, 2, D], BF16, kind="ExternalInput")
    ebiasT = nc.dram_tensor("ebiasT", [N, N], BF16, kind="ExternalInput")
    bv_r = nc.dram_tensor("bv_r", [1, DSL], BF16, kind="ExternalInput")
    bq_col = nc.dram_tensor("bq_col", [128, 2], F32, kind="ExternalInput")
    bk_col = nc.dram_tensor("bk_col", [128, 2], F32, kind="ExternalInput")
    scol = nc.dram_tensor("scol", [128, MT], F32, kind="ExternalInput")
    ones64 = nc.dram_tensor("ones64", [65, 64], F32R, kind="ExternalInput")
    onescol = nc.dram_tensor("onescol", [128, 1], BF16, kind="ExternalInput")
    onesr = nc.dram_tensor("onesr", [1, 128], BF16, kind="ExternalInput")
    identb = nc.dram_tensor("identb", [128, 128], BF16, kind="ExternalInput")
    out_part = nc.dram_tensor("out_part", [N, D], BF16, kind="ExternalOutput")

    with tile.TileContext(nc) as tc:
        with tc.tile_pool(name="consts", bufs=1) as consts, \
             tc.tile_pool(name="persist", bufs=1) as persist, \
             tc.tile_pool(name="stream", bufs=4) as stream, \
             tc.tile_pool(name="work", bufs=3) as work, \
             tc.tile_pool(name="outp", bufs=2) as outp, \
             tc.tile_pool(name="psum", bufs=1, space="PSUM") as pp:

            # ---- constants -------------------------------------------------
            xT_sb = consts.tile([128, 4, 8, 512], BF16, tag="xT")
            wq_t = consts.tile([128, 2, 8, 128], BF16, tag="wq")
            wk_t = consts.tile([128, 2, 8, 128], BF16, tag="wk")
            wv_t = consts.tile([128, 8, DSL], BF16, tag="wv")
            wo_t = consts.tile([128, 2, D], BF16, tag="wo")
            identb_t = consts.tile([128, 128], BF16, tag="identb")
            bv_t = consts.tile([1, DSL], BF16, tag="bv")
            bq_c = consts.tile([128, 2], F32, tag="bqc")
            bk_c = consts.tile([128, 2], F32, tag="bkc")
            scol_t = consts.tile([128, MT], F32, tag="scol")
            ones64_t = consts.tile([65, 64], F32R, tag="ones64")
            onescol_t = consts.tile([128, 1], BF16, tag="onescol")
            onesr_t = consts.tile([1, 128], BF16, tag="onesr")
            # DMA order = arrival order (~175 GB/s effective, ~9us startup).
            nc.sync.dma_start(out=identb_t, in_=identb[:])
            nc.sync.dma_start(out=onescol_t, in_=onescol[:])
            nc.sync.dma_start(out=onesr_t, in_=onesr[:])
            nc.sync.dma_start(out=bq_c, in_=bq_col[:])
            nc.sync.dma_start(out=bk_c, in_=bk_col[:])
            nc.sync.dma_start(out=scol_t, in_=scol[:])
            nc.sync.dma_start(out=xT_sb[:, 0], in_=xT[:, 0])
            nc.sync.dma_start(out=wk_t[:, 0], in_=wk2[:, 0])
            nc.sync.dma_start(out=wq_t[:, 0], in_=wq2[:, 0])
            nc.sync.dma_start(out=wv_t, in_=wv[:])
            nc.sync.dma_start(out=bv_t, in_=bv_r[:])
            nc.sync.dma_start(out=ones64_t, in_=ones64[:])

            # ---- persistent intermediates ---------------------------------
            # q^T/k^T: [dk-pair row hi*64+dk, head-pair hp, n]; kT UNSCALED
            qT_all = persist.tile([128, 2, N], BF16, tag="qT")
            kT_all = persist.tile([128, 2, N], BF16, tag="kT")
            # v (scaled) + ones col: [m-part, m-tile, head, 65]
            vaug = persist.tile([128, MT, HPC, 65], BF16, tag="vaug")
            # normalized y^T for O-proj
            yT_all = persist.tile([128, 2, N], BF16, tag="yT")
            # per-quarter y + denominators staging
            y_sb = persist.tile([65, HPC, 512], F32R, tag="ysb")

            # vaug ones columns, written once
            nc.vector.tensor_copy(
                vaug[:, :, :, 64:65],
                onescol_t.unsqueeze(1).unsqueeze(1)
                .broadcast_to([128, MT, HPC, 1]))

            state = {}

            def eb_load(q4, mt):
                eb_t = stream.tile([128, 512], BF16, tag="ebias", bufs=18,
                                   name=f"b{q4}_{mt}")
                nc.sync.dma_start(
                    out=eb_t,
                    in_=ebiasT[mt * 128:mt * 128 + 128,
                               q4 * 512:q4 * 512 + 512])
                state[("eb", q4, mt)] = eb_t

            # prefetch ALL q0 bias tiles, interleaved with the remaining
            # x blocks in exact deadline order: kproj_h(1,0) at round ~2
            # needs xT1 almost immediately, so only two bias tiles go
            # ahead of it; later blocks have progressively more slack.
            nc.sync.dma_start(out=xT_sb[:, 1], in_=xT[:, 1])
            eb_load(0, 0)
            eb_load(0, 1)
            eb_load(0, 2)
            eb_load(0, 3)
            nc.sync.dma_start(out=xT_sb[:, 2], in_=xT[:, 2])
            for mt in range(4, 10):
                eb_load(0, mt)
            nc.sync.dma_start(out=xT_sb[:, 3], in_=xT[:, 3])
            for mt in range(10, 16):
                eb_load(0, mt)
            nc.sync.dma_start(out=wk_t[:, 1], in_=wk2[:, 1])
            nc.sync.dma_start(out=wq_t[:, 1], in_=wq2[:, 1])
            nc.sync.dma_start(out=wo_t, in_=wo[:])

            # ---- projections (small s-tag PSUM, interleaved into rounds) --
            def kproj_h(c, hp):
                nsl = slice(c * 512, c * 512 + 512)
                ps = pp.tile([128, 512], F32, tag="s", bufs=2,
                             name=f"kps{c}_{hp}")
                for t in range(8):
                    nc.tensor.matmul(
                        ps, wk_t[:, hp, t, :], xT_sb[:, c, t, :],
                        start=(t == 0), stop=(t == 7))
                nc.vector.tensor_scalar(
                    kT_all[:, hp, nsl], ps, bk_c[:, hp:hp + 1], None, Add)

            def qproj_h(c, hp):
                nsl = slice(c * 512, c * 512 + 512)
                ps = pp.tile([128, 512], F32, tag="s", bufs=2,
                             name=f"qps{c}_{hp}")
                for t in range(8):
                    nc.tensor.matmul(
                        ps, wq_t[:, hp, t, :], xT_sb[:, c, t, :],
                        start=(t == 0), stop=(t == 7))
                nc.vector.tensor_scalar(
                    qT_all[:, hp, nsl], ps,
                    0.125, bq_c[:, hp:hp + 1], Mult, Add)

            def vproj(mt):
                mb, mo = divmod(mt, 4)
                vp = pp.tile([128, 256], F32, tag="s", bufs=2,
                             name=f"vps{mt}")
                for t in range(8):
                    nc.tensor.matmul(
                        vp, xT_sb[:, mb, t, mo * 128:mo * 128 + 128],
                        wv_t[:, t, :], start=(t == 0), stop=False)
                nc.tensor.matmul(
                    vp, onesr_t[0:1, :], bv_t[0:1, :], start=False, stop=True)
                vr = vp.rearrange("p (h d) -> p h d", h=HPC)
                nc.vector.tensor_scalar(
                    vaug[:, mt, :, 0:64], vr,
                    scol_t[:, mt:mt + 1], None, Mult)

            warm = pp.tile([128, 512], F32, tag="s", bufs=2, name="warm")
            for w in range(52):
                nc.tensor.matmul(warm[:, 0:128], identb_t, identb_t,
                                 start=(w == 0), stop=(w == 51))

            # minimum to start round 0 of pass (0,0): kT/qT chunk 0 only.
            # vproj(0..) rides the round inserts -- AV can lag a few rounds
            # behind the exp cadence (a/e bufs below give the headroom).
            kproj_h(0, 0)
            qproj_h(0, 0)

            # ---- round bodies ---------------------------------------------
            def qk_round(q4, p, mt):
                nsl = slice(q4 * 512, q4 * 512 + 512)
                if p == 0 and ("eb", q4, mt) not in state:
                    eb_load(q4, mt)
                eb_t = state[("eb", q4, mt)]
                if p == 1:
                    del state[("eb", q4, mt)]
                s_ps = pp.tile([128, 2, 512], F32, tag="s2", bufs=2,
                               name=f"s{q4}_{p}_{mt}")
                for hi in range(2):
                    nc.tensor.matmul(
                        s_ps[:, hi],
                        kT_all[hi * 64:hi * 64 + 64, p,
                               mt * 128:mt * 128 + 128],
                        qT_all[hi * 64:hi * 64 + 64, p, nsl],
                        start=True, stop=True)
                e_t = work.tile([128, 2, 512], BF16, tag="e", bufs=6,
                                name=f"e{q4}_{p}_{mt}")
                # per-key-partition (1+alpha) scale rides the exp
                nc.scalar.activation(e_t, s_ps, Exp,
                                     scale=scol_t[:, mt:mt + 1])
                a_t = work.tile([128, 2, 512], BF16, tag="a", bufs=6,
                                name=f"a{q4}_{p}_{mt}")
                nc.vector.tensor_mul(
                    a_t, e_t,
                    eb_t.unsqueeze(1).broadcast_to([128, 2, 512]))
                state[("a", mt % 6)] = a_t

            def av_round(q4, p, mt):
                a_t = state[("a", mt % 6)]
                y_ps = state["y"]
                for hi in range(2):
                    nc.tensor.matmul(
                        y_ps[:, hi], vaug[:, mt, 2 * p + hi, :], a_t[:, hi],
                        start=(mt == 0), stop=(mt == MT - 1))

            def tail_b(q4, hq):
                # one head-pair: 2 recip-broadcast matmuls (PE) + 2 muls (DVE)
                r_row = state[("rrow", q4, hq)]
                for hi in range(2):
                    h = hq * 2 + hi
                    rb = pp.tile([128, 512], F32, tag="s", bufs=2,
                                 name=f"rb{q4}_{h}")
                    nc.tensor.matmul(
                        rb[0:64, :], ones64_t[0:1, :],
                        r_row[0:1, hi, :], start=True, stop=True)
                    nc.vector.tensor_mul(
                        yT_all[hi * 64:hi * 64 + 64, hq,
                               q4 * 512:q4 * 512 + 512],
                        y_sb[0:64, h, :].bitcast(F32), rb[0:64, :])

            def oproj_full(q4, j, tail=False):
                # both dc halves together: the yT stationary operand is
                # shared by consecutive matmuls, so with ldw-opt walrus
                # loads it once per hp instead of once per matmul.
                nt = q4 * 4 + j
                o_ps = [pp.tile([128, 512], F32, tag="s", bufs=2,
                                name=f"o{nt}_{dc}") for dc in range(2)]
                for hp in range(2):
                    for dc in range(2):
                        nc.tensor.matmul(
                            o_ps[dc],
                            yT_all[:, hp, nt * 128:nt * 128 + 128],
                            wo_t[:, hp, dc * 512:dc * 512 + 512],
                            start=(hp == 0), stop=(hp == 1))
                for dc in range(2):
                    o_sb = outp.tile([128, 512], BF16, tag="osb",
                                     name=f"ob{nt}_{dc}")
                    # in the final tail ACT is idle: alternate copy engines
                    if tail and dc == 1:
                        nc.scalar.copy(o_sb, o_ps[dc])
                    else:
                        nc.vector.tensor_copy(o_sb, o_ps[dc])
                    # gpsimd queue: keeps compute-gated stores from head-of-
                    # line blocking the ebias loads on the sync queue
                    (nc.sync if tail and dc == 1 else nc.gpsimd).dma_start(
                        out=out_part[nt * 128:nt * 128 + 128,
                                     dc * 512:dc * 512 + 512], in_=o_sb)

            # insertion schedules: {(q4==0, p): {mt: [fns]}} built inline
            def extra(q4, p, mt):
                if q4 == 0 and p == 0:
                    sched = {
                        1: [lambda: vproj(1)],
                        2: [lambda: kproj_h(1, 0), lambda: vproj(2)],
                        3: [lambda: vproj(3)], 4: [lambda: vproj(4)],
                        5: [lambda: vproj(5)],
                        6: [lambda: kproj_h(2, 0), lambda: vproj(6)],
                        7: [lambda: vproj(7)], 8: [lambda: vproj(8)],
                        9: [lambda: vproj(9)],
                        10: [lambda: kproj_h(3, 0), lambda: vproj(10)],
                        11: [lambda: vproj(11)], 12: [lambda: vproj(12)],
                        13: [lambda: vproj(13)],
                        14: [lambda: vproj(14), lambda: qproj_h(0, 1)],
                        15: [lambda: vproj(15), lambda: kproj_h(0, 1)],
                    }
                elif q4 == 0 and p == 1:
                    sched = {
                        2: [lambda: kproj_h(1, 1)],
                        4: [lambda: kproj_h(2, 1), lambda: qproj_h(1, 0)],
                        6: [lambda: kproj_h(3, 1), lambda: qproj_h(1, 1)],
                        10: [lambda: tail_b(0, 0)],
                        13: [lambda: eb_load(1, 0)],
                        14: [lambda: eb_load(1, 1)],
                    }
                elif p == 0:
                    sched = {8: [lambda: tail_b(q4 - 1, 1)]}
                    for k in range(4):
                        sched[9 + 2 * k] = [
                            (lambda jj: lambda: oproj_full(q4 - 1, jj))(k)]
                else:
                    sched = {
                        10: [lambda: tail_b(q4, 0)],
                    }
                    if q4 < NQ4 - 1:
                        sched[4] = [lambda: qproj_h(q4 + 1, 0)]
                        sched[6] = [lambda: qproj_h(q4 + 1, 1)]
                        sched[13] = [lambda: eb_load(q4 + 1, 0)]
                        sched[14] = [lambda: eb_load(q4 + 1, 1)]
                for fn in sched.get(mt, ()):
                    fn()

            # ---- main pass loop -------------------------------------------
            for q4 in range(NQ4):
                for p in range(2):
                    state["y"] = pp.tile([65, 2, 512], F32, tag="y", bufs=1,
                                         name=f"y{q4}_{p}")
                    qk_round(q4, p, 0)
                    if q4 == 0 and p == 0:
                        vproj(0)
                    for mt in range(1, MT):
                        qk_round(q4, p, mt)
                        av_round(q4, p, mt - 1)
                        extra(q4, p, mt)
                    av_round(q4, p, MT - 1)
                    y_ps = state.pop("y")
                    nc.vector.tensor_copy(
                        y_sb[:, 2 * p:2 * p + 2, :], y_ps)
                    # denominators for this head pair: direct reciprocal on
                    # the single-partition accumulator row (no DMA reshape)
                    # reshape the denom row onto 32 partitions (32 DMA
                    # descriptors), cheap 32-lane reciprocal, DMA back to a
                    # row.  Consumers (tail_b) are scheduled 10+ rounds out
                    # so the descriptor latency is hidden.
                    d_t = work.tile([32, 32], F32R, tag="dt", bufs=2,
                                    name=f"dt{q4}_{p}")
                    # gpsimd queue: this DMA waits on the y evacuation, and
                    # on the sync queue that wait head-of-line blocked the
                    # next pass's first bias-tile loads (~2us stalls at mt3)
                    nc.gpsimd.dma_start(
                        out=d_t, in_=y_sb[64:65, 2 * p:2 * p + 2, :])
                    d_r = work.tile([32, 32], F32R, tag="dr", bufs=2,
                                    name=f"dr{q4}_{p}")
                    nc.vector.reciprocal(out=d_r.bitcast(F32),
                                         in_=d_t.bitcast(F32))
                    r_row = work.tile([1, 2, 512], F32R, tag="rrow", bufs=2,
                                      name=f"rr{q4}_{p}")
                    nc.gpsimd.dma_start(out=r_row, in_=d_r)
                    state[("rrow", q4, p)] = r_row

            # final quarter tail: bridge with keep-warm matmuls, then the
            # last normalize + O-proj.
            warm2 = pp.tile([128, 512], F32, tag="s", bufs=2, name="warm2")
            for w in range(36):
                nc.tensor.matmul(warm2, identb_t, kT_all[:, 0, 0:512],
                                 start=(w == 0), stop=(w == 35))
            tail_b(NQ4 - 1, 1)
            for j in range(4):
                oproj_full(NQ4 - 1, j, tail=True)

    return nc


def _ensure_ntff_hook():
    """Register the axon NTFF profiling hook if the agent image lacks
    antenv.axon_hooks (profiling only; kernel runs fine without)."""
    try:
        from antenv.axon_hooks import get_axon_ntff_profile_hook  # noqa: F401
        return
    except ImportError:
        pass
    import types
    import antenv
    from trn_agent_boot.trn_boot import _ntff_profile_via_ctypes

    mod = types.ModuleType("antenv.axon_hooks")
    holder = {}
    mod.set_axon_ntff_profile_hook = lambda h: holder.__setitem__("h", h)
    mod.get_axon_ntff_profile_hook = lambda: holder.get("h")
    sys.modules["antenv.axon_hooks"] = mod
    antenv.axon_hooks = mod
    mod.set_axon_ntff_profile_hook(
        _ntff_profile_via_ctypes("/opt/axon/libaxon_pjrt.so"))


_NC_CACHE: dict = {}


def _get_nc() -> bass.Bass:
    if "nc" not in _NC_CACHE:
        _NC_CACHE["nc"] = _build_nc()
    return _NC_CACHE["nc"]


def _col_layout(v):
    """[256] per-core head-slice -> [128, 2] f32: row (h%2)*64+dk, col h//2."""
    return np.ascontiguousarray(
        v.reshape(2, 2, 64).transpose(1, 2, 0).reshape(128, 2)
    ).astype(np.float32)


def _w_hp_layout(w, bf):
    """W[:, dsl] (1024, 256) -> [128, 2(hp), 8(t), 128] bf16."""
    a = w.astype(bf).reshape(8, 128, 2, 128)     # [t, part, hp, col]
    return np.ascontiguousarray(a.transpose(1, 2, 0, 3))


def kernel(x, alpha, bias, Wq, bq, Wk, bk, Wv, bv, Wo, bo, trace=False):
    bf = ml_dtypes.bfloat16
    x = np.asarray(x, np.float32)
    alpha = np.asarray(alpha, np.float32)
    bias = np.asarray(bias, np.float32)
    Wq = np.asarray(Wq, np.float32); bq = np.asarray(bq, np.float32)
    Wk = np.asarray(Wk, np.float32); bk = np.asarray(bk, np.float32)
    Wv = np.asarray(Wv, np.float32); bv = np.asarray(bv, np.float32)
    Wo = np.asarray(Wo, np.float32); bo = np.asarray(bo, np.float32)

    c = np.ascontiguousarray

    in_maps = []
    per_b = {}
    for b in range(B):
        s = 1.0 + alpha[b]                             # (N,)
        xt = x[b].T.astype(bf)                         # (D, N)
        per_b[b] = {
            # SBUF layout [p, block, t, 512] -> contiguous 8KB block rows
            "xT": c(xt.reshape(8, 128, 4, 512).transpose(1, 2, 0, 3)),
            "ebiasT": c(np.exp(bias[b].T).astype(bf)),  # (N, N) [m, n]
            # (1+alpha) laid out per key partition: [p, mt] = s[mt*128+p]
            "scol": c(s.reshape(MT, 128).T.astype(np.float32)),
        }
    for core in range(NCORES):
        b, hg = divmod(core, 4)
        dsl = slice(hg * DSL, hg * DSL + DSL)
        in_maps.append({
            **per_b[b],
            "wq2": _w_hp_layout(Wq[:, dsl], bf),
            "wk2": _w_hp_layout(Wk[:, dsl], bf),
            "wv": c(Wv[:, dsl].astype(bf).reshape(8, 128, DSL).transpose(1, 0, 2)),
            "wo": c(Wo[dsl, :].astype(bf).reshape(2, 128, D).transpose(1, 0, 2)),
            "bv_r": c(bv[dsl].reshape(1, DSL).astype(bf)),
            "bq_col": _col_layout(0.125 * bq[dsl]),
            "bk_col": _col_layout(bk[dsl]),
            "ones64": np.ones((65, 64), np.float32),
            "onescol": np.ones((128, 1), bf),
            "onesr": np.ones((1, 128), bf),
            "identb": np.eye(128, dtype=bf),
        })

    if trace:
        _ensure_ntff_hook()
    nc = _get_nc()
    res = run_bass_kernel_spmd(
        nc, in_maps, core_ids=list(range(NCORES)), trace=trace)

    out = np.zeros((B, N, D), np.float32)
    for core in range(NCORES):
        out[core // 4] += res.results[core]["out_part"].astype(np.float32)
    out += bo[None, None, :]
    if trace:
        kernel.last_exec_time_ns = res.exec_time_ns
        kernel.last_profile = res.profile_json
    return out


# revision 28
# speedup vs baseline: 1.0036x; 1.0008x over previous
"""Biased self-attention TRN2 Bass kernel (8 NeuronCores), v6.

Problem: nn_BiasedSelfAttention — B=2, N=2048, D=1024, H=16, DK=64.
    q,k,v = split_heads(x@Wq+bq), ...; k,v scaled by (1+alpha[b,n]);
    logits = q k^T/sqrt(DK) + bias[b][None]; y = softmax(logits) v;
    out = merge_heads(y) @ Wo + bo.

Sharding: 8 cores = (batch b in {0,1}) x (head-group hg in {0..3} of 4
heads = 256 dims of D).  Data parallel over B, tensor parallel over H.
Each core computes a partial O-projection (its 256 rows of Wo); the
host sums the 4 partials per batch and adds bo.

v6 design (v5 profile: rounds fully ACT-bound at 1325ns because the
FD=512 exp pays ~260ns/instr overhead twice; tail_b at round 4 of each
quarter blocked the PE FIFO ~7us on the reciprocal's SBUF->SBUF DMA
chain; PSUM had no spare banks to decouple projections from QK):
  - quarters split into two 16-round PASSES (head pair = pass).  PSUM:
    qk s-tag [128,2,512] x2 (4 banks) + y [65,2,512] (2 banks) + small
    s-tag x2 (2 banks) = 8.  Projections/O-proj/tail use the small tag
    so they never steal the QK double-buffer.
  - exp is ONE FD=1024 ACTIVATE per round again (PSUM source, per-key
    (1+alpha) scale AP); DVE bf16 2x multiply by host-precomputed
    exp(bias); ebias tiles are loaded once per quarter and reused by
    both passes.
  - denominators: per-pass DVE reciprocal directly on the single-
    partition accumulator row -- the DMA reshape chain is gone.
  - k/q weights are loaded in hp halves so round 0 starts ~21us in.
"""

import json
import sys

sys.path.insert(0, "/opt/trn_rl_repo")

import numpy as np
import ml_dtypes

import concourse.bass as bass
import concourse.mybir as mybir
import concourse.tile as tile
from concourse.bass_utils import run_bass_kernel_spmd

# ---------------------------------------------------------------- bir fix --
# The pinned walrus encodes at most ONE sem-wait per instruction, but Tile's
# wait-assigner can emit several.  Hoist extras onto EventSemaphore
# instructions just before the instruction.


def _split_multi_waits(bir_json: bytes) -> bytes:
    m = json.loads(bir_json)
    for fn in m.get("functions", []):
        for blk in fn.get("blocks", []):
            insts = blk.get("instructions")
            if not insts:
                continue
            out = []
            for inst in insts:
                sync = inst.get("sync_info")
                waits = (sync or {}).get("on_wait") or []
                if len(waits) > 1:
                    for i, w in enumerate(waits[:-1]):
                        out.append({
                            "debug": inst.get("debug", 0),
                            "engine": inst["engine"],
                            "ins": [],
                            "name": f"{inst['name']}-sw{i}",
                            "opcode": "EventSemaphore",
                            "outs": [],
                            "sync_info": {"on_update": [], "on_wait": [w]},
                        })
                    sync["on_wait"] = waits[-1:]
                out.append(inst)
            blk["instructions"] = out
    return json.dumps(m).encode()


def _patch_bass():
    if getattr(bass.Bass, "_multiwait_patched", False):
        return
    orig = bass.Bass.to_json_bytes

    def to_json_bytes(self, *a, **kw):
        return _split_multi_waits(orig(self, *a, **kw))

    bass.Bass.to_json_bytes = to_json_bytes
    bass.Bass._multiwait_patched = True


_patch_bass()


def _patch_ldw_opt():
    """Enable walrus's redundant-LDWEIGHTS elimination (off by default in
    bass_utils).  Consecutive matmuls that share a stationary operand then
    load it once."""
    import concourse.bass_utils as _bu
    if getattr(_bu, "_ldw_opt_patched", False):
        return
    orig = _bu.run_command

    def run_command(cmd, *a, **kw):
        # ldw-opt=true crashes walrus codegen (visitInstLdweights) on this
        # pinned compiler -- keep the flag off; wrapper retained as a hook.
        return orig(cmd, *a, **kw)

    _bu.run_command = run_command
    _bu._ldw_opt_patched = True


_patch_ldw_opt()

# ------------------------------------------------------------- dimensions --
B, N, D, H = 2, 2048, 1024, 16
DK = D // H                      # 64
NCORES = 8
HPC = H // 4                     # 4 heads per core
DSL = HPC * DK                   # 256 D-columns per core
NQ4 = N // 512                   # 4 query quarters
MT = N // 128                    # 16 key tiles
F32 = mybir.dt.float32
F32R = mybir.dt.float32r
BF16 = mybir.dt.bfloat16
Exp = mybir.ActivationFunctionType.Exp
Copy = mybir.ActivationFunctionType.Copy
Add = mybir.AluOpType.add
Mult = mybir.AluOpType.mult


def _build_nc() -> bass.Bass:
    nc = bass.Bass()

    xT = nc.dram_tensor("xT", [128, 4, 8, 512], BF16, kind="ExternalInput")
    wq2 = nc.dram_tensor("wq2", [128, 2, 8, 128], BF16, kind="ExternalInput")
    wk2 = nc.dram_tensor("wk2", [128, 2, 8, 128], BF16, kind="ExternalInput")
    wv = nc.dram_tensor("wv", [128, 8, DSL], BF16, kind="ExternalInput")
    wo = nc.dram_tensor("wo", [128, 2, D], BF16, kind="ExternalInput")
    ebiasT = nc.dram_tensor("ebiasT", [N, N], BF16, kind="ExternalInput")
    bv_r = nc.dram_tensor("bv_r", [1, DSL], BF16, kind="ExternalInput")
    bq_col = nc.dram_tensor("bq_col", [128, 2], F32, kind="ExternalInput")
    bk_col = nc.dram_tensor("bk_col", [128, 2], F32, kind="ExternalInput")
    scol = nc.dram_tensor("scol", [128, MT], F32, kind="ExternalInput")
    ones64 = nc.dram_tensor("ones64", [65, 64], F32R, kind="ExternalInput")
    onescol = nc.dram_tensor("onescol", [128, 1], BF16, kind="ExternalInput")
    onesr = nc.dram_tensor("onesr", [1, 128], BF16, kind="ExternalInput")
    identb = nc.dram_tensor("identb", [128, 128], BF16, kind="ExternalInput")
    out_part = nc.dram_tensor("out_part", [N, D], BF16, kind="ExternalOutput")

    with tile.TileContext(nc) as tc:
        with tc.tile_pool(name="consts", bufs=1) as consts, \
             tc.tile_pool(name="persist", bufs=1) as persist, \
             tc.tile_pool(name="stream", bufs=4) as stream, \
             tc.tile_pool(name="work", bufs=3) as work, \
             tc.tile_pool(name="outp", bufs=2) as outp, \
             tc.tile_pool(name="psum", bufs=1, space="PSUM") as pp:

            # ---- constants -------------------------------------------------
            xT_sb = consts.tile([128, 4, 8, 512], BF16, tag="xT")
            wq_t = consts.tile([128, 2, 8, 128], BF16, tag="wq")
            wk_t = consts.tile([128, 2, 8, 128], BF16, tag="wk")
            wv_t = consts.tile([128, 8, DSL], BF16, tag="wv")
            wo_t = consts.tile([128, 2, D], BF16, tag="wo")
            identb_t = consts.tile([128, 128], BF16, tag="identb")
            bv_t = consts.tile([1, DSL], BF16, tag="bv")
            bq_c = consts.tile([128, 2], F32, tag="bqc")
            bk_c = consts.tile([128, 2], F32, tag="bkc")
            scol_t = consts.tile([128, MT], F32, tag="scol")
            ones64_t = consts.tile([65, 64], F32R, tag="ones64")
            onescol_t = consts.tile([128, 1], BF16, tag="onescol")
            onesr_t = consts.tile([1, 128], BF16, tag="onesr")
            # DMA order = arrival order (~175 GB/s effective, ~9us startup).
            nc.sync.dma_start(out=identb_t, in_=identb[:])
            nc.sync.dma_start(out=onescol_t, in_=onescol[:])
            nc.sync.dma_start(out=onesr_t, in_=onesr[:])
            nc.sync.dma_start(out=bq_c, in_=bq_col[:])
            nc.sync.dma_start(out=bk_c, in_=bk_col[:])
            nc.sync.dma_start(out=scol_t, in_=scol[:])
            nc.sync.dma_start(out=xT_sb[:, 0], in_=xT[:, 0])
            nc.sync.dma_start(out=wk_t[:, 0], in_=wk2[:, 0])
            nc.sync.dma_start(out=wq_t[:, 0], in_=wq2[:, 0])
            nc.sync.dma_start(out=wv_t, in_=wv[:])
            nc.sync.dma_start(out=bv_t, in_=bv_r[:])
            nc.sync.dma_start(out=ones64_t, in_=ones64[:])

            # ---- persistent intermediates ---------------------------------
            # q^T/k^T: [dk-pair row hi*64+dk, head-pair hp, n]; kT UNSCALED
            qT_all = persist.tile([128, 2, N], BF16, tag="qT")
            kT_all = persist.tile([128, 2, N], BF16, tag="kT")
            # v (scaled) + ones col: [m-part, m-tile, head, 65]
            vaug = persist.tile([128, MT, HPC, 65], BF16, tag="vaug")
            # normalized y^T for O-proj
            yT_all = persist.tile([128, 2, N], BF16, tag="yT")
            # per-quarter y + denominators staging
            y_sb = persist.tile([65, HPC, 512], F32R, tag="ysb")

            # vaug ones columns, written once
            nc.vector.tensor_copy(
                vaug[:, :, :, 64:65],
                onescol_t.unsqueeze(1).unsqueeze(1)
                .broadcast_to([128, MT, HPC, 1]))

            state = {}

            def eb_load(q4, mt):
                eb_t = stream.tile([128, 512], BF16, tag="ebias", bufs=18,
                                   name=f"b{q4}_{mt}")
                nc.sync.dma_start(
                    out=eb_t,
                    in_=ebiasT[mt * 128:mt * 128 + 128,
                               q4 * 512:q4 * 512 + 512])
                state[("eb", q4, mt)] = eb_t

            # prefetch ALL q0 bias tiles, interleaved with the remaining
            # x blocks in exact deadline order: kproj_h(1,0) at round ~2
            # needs xT1 almost immediately, so only two bias tiles go
            # ahead of it; later blocks have progressively more slack.
            nc.sync.dma_start(out=xT_sb[:, 1], in_=xT[:, 1])
            eb_load(0, 0)
            eb_load(0, 1)
            eb_load(0, 2)
            eb_load(0, 3)
            nc.sync.dma_start(out=xT_sb[:, 2], in_=xT[:, 2])
            for mt in range(4, 10):
                eb_load(0, mt)
            nc.sync.dma_start(out=xT_sb[:, 3], in_=xT[:, 3])
            for mt in range(10, 16):
                eb_load(0, mt)
            nc.sync.dma_start(out=wk_t[:, 1], in_=wk2[:, 1])
            nc.sync.dma_start(out=wq_t[:, 1], in_=wq2[:, 1])
            nc.sync.dma_start(out=wo_t, in_=wo[:])

            # ---- projections (small s-tag PSUM, interleaved into rounds) --
            def kproj_h(c, hp):
                nsl = slice(c * 512, c * 512 + 512)
                ps = pp.tile([128, 512], F32, tag="s", bufs=2,
                             name=f"kps{c}_{hp}")
                for t in range(8):
                    nc.tensor.matmul(
                        ps, wk_t[:, hp, t, :], xT_sb[:, c, t, :],
                        start=(t == 0), stop=(t == 7))
                nc.vector.tensor_scalar(
                    kT_all[:, hp, nsl], ps, bk_c[:, hp:hp + 1], None, Add)

            def qproj_h(c, hp):
                nsl = slice(c * 512, c * 512 + 512)
                ps = pp.tile([128, 512], F32, tag="s", bufs=2,
                             name=f"qps{c}_{hp}")
                for t in range(8):
                    nc.tensor.matmul(
                        ps, wq_t[:, hp, t, :], xT_sb[:, c, t, :],
                        start=(t == 0), stop=(t == 7))
                nc.vector.tensor_scalar(
                    qT_all[:, hp, nsl], ps,
                    0.125, bq_c[:, hp:hp + 1], Mult, Add)

            def vproj(mt):
                mb, mo = divmod(mt, 4)
                vp = pp.tile([128, 256], F32, tag="s", bufs=2,
                             name=f"vps{mt}")
                for t in range(8):
                    nc.tensor.matmul(
                        vp, xT_sb[:, mb, t, mo * 128:mo * 128 + 128],
                        wv_t[:, t, :], start=(t == 0), stop=False)
                nc.tensor.matmul(
                    vp, onesr_t[0:1, :], bv_t[0:1, :], start=False, stop=True)
                vr = vp.rearrange("p (h d) -> p h d", h=HPC)
                nc.vector.tensor_scalar(
                    vaug[:, mt, :, 0:64], vr,
                    scol_t[:, mt:mt + 1], None, Mult)

            warm = pp.tile([128, 512], F32, tag="s", bufs=2, name="warm")
            for w in range(52):
                nc.tensor.matmul(warm[:, 0:128], identb_t, identb_t,
                                 start=(w == 0), stop=(w == 51))

            # minimum to start round 0 of pass (0,0): kT/qT chunk 0 only.
            # vproj(0..) rides the round inserts -- AV can lag a few rounds
            # behind the exp cadence (a/e bufs below give the headroom).
            kproj_h(0, 0)
            qproj_h(0, 0)

            # ---- round bodies ---------------------------------------------
            def qk_round(q4, p, mt):
                nsl = slice(q4 * 512, q4 * 512 + 512)
                if p == 0 and ("eb", q4, mt) not in state:
                    eb_load(q4, mt)
                eb_t = state[("eb", q4, mt)]
                if p == 1:
                    del state[("eb", q4, mt)]
                s_ps = pp.tile([128, 2, 512], F32, tag="s2", bufs=2,
                               name=f"s{q4}_{p}_{mt}")
                for hi in range(2):
                    nc.tensor.matmul(
                        s_ps[:, hi],
                        kT_all[hi * 64:hi * 64 + 64, p,
                               mt * 128:mt * 128 + 128],
                        qT_all[hi * 64:hi * 64 + 64, p, nsl],
                        start=True, stop=True)
                e_t = work.tile([128, 2, 512], BF16, tag="e", bufs=6,
                                name=f"e{q4}_{p}_{mt}")
                # per-key-partition (1+alpha) scale rides the exp
                nc.scalar.activation(e_t, s_ps, Exp,
                                     scale=scol_t[:, mt:mt + 1])
                a_t = work.tile([128, 2, 512], BF16, tag="a", bufs=6,
                                name=f"a{q4}_{p}_{mt}")
                nc.vector.tensor_mul(
                    a_t, e_t,
                    eb_t.unsqueeze(1).broadcast_to([128, 2, 512]))
                state[("a", mt % 6)] = a_t

            def av_round(q4, p, mt):
                a_t = state[("a", mt % 6)]
                y_ps = state["y"]
                for hi in range(2):
                    nc.tensor.matmul(
                        y_ps[:, hi], vaug[:, mt, 2 * p + hi, :], a_t[:, hi],
                        start=(mt == 0), stop=(mt == MT - 1))

            def tail_b(q4, hq):
                # one head-pair: 2 recip-broadcast matmuls (PE) + 2 muls (DVE)
                r_row = state[("rrow", q4, hq)]
                for hi in range(2):
                    h = hq * 2 + hi
                    rb = pp.tile([128, 512], F32, tag="s", bufs=2,
                                 name=f"rb{q4}_{h}")
                    nc.tensor.matmul(
                        rb[0:64, :], ones64_t[0:1, :],
                        r_row[0:1, hi, :], start=True, stop=True)
                    nc.vector.tensor_mul(
                        yT_all[hi * 64:hi * 64 + 64, hq,
                               q4 * 512:q4 * 512 + 512],
                        y_sb[0:64, h, :].bitcast(F32), rb[0:64, :])

            def oproj_full(q4, j, tail=False):
                # both dc halves together: the yT stationary operand is
                # shared by consecutive matmuls, so with ldw-opt walrus
                # loads it once per hp instead of once per matmul.
                nt = q4 * 4 + j
                o_ps = [pp.tile([128, 512], F32, tag="s", bufs=2,
                                name=f"o{nt}_{dc}") for dc in range(2)]
                for hp in range(2):
                    for dc in range(2):
                        nc.tensor.matmul(
                            o_ps[dc],
                            yT_all[:, hp, nt * 128:nt * 128 + 128],
                            wo_t[:, hp, dc * 512:dc * 512 + 512],
                            start=(hp == 0), stop=(hp == 1))
                for dc in range(2):
                    o_sb = outp.tile([128, 512], BF16, tag="osb",
                                     name=f"ob{nt}_{dc}")
                    # in the final tail ACT is idle: alternate copy engines
                    if tail and dc == 1:
                        nc.scalar.copy(o_sb, o_ps[dc])
                    else:
                        nc.vector.tensor_copy(o_sb, o_ps[dc])
                    # gpsimd queue: keeps compute-gated stores from head-of-
                    # line blocking the ebias loads on the sync queue
                    (nc.sync if tail and dc == 1 else nc.gpsimd).dma_start(
                        out=out_part[nt * 128:nt * 128 + 128,
                                     dc * 512:dc * 512 + 512], in_=o_sb)

            # insertion schedules: {(q4==0, p): {mt: [fns]}} built inline
            def extra(q4, p, mt):
                if q4 == 0 and p == 0:
                    sched = {
                        1: [lambda: vproj(1)],
                        2: [lambda: kproj_h(1, 0), lambda: vproj(2)],
                        3: [lambda: vproj(3)], 4: [lambda: vproj(4)],
                        5: [lambda: vproj(5)],
                        6: [lambda: kproj_h(2, 0), lambda: vproj(6)],
                        7: [lambda: vproj(7)], 8: [lambda: vproj(8)],
                        9: [lambda: vproj(9)],
                        10: [lambda: kproj_h(3, 0), lambda: vproj(10)],
                        11: [lambda: vproj(11)],
                        12: [lambda: vproj(12), lambda: qproj_h(0, 1)],
                        13: [lambda: vproj(13), lambda: kproj_h(0, 1)],
                        14: [lambda: vproj(14)],
                        15: [lambda: vproj(15)],
                    }
                elif q4 == 0 and p == 1:
                    sched = {
                        2: [lambda: kproj_h(1, 1)],
                        4: [lambda: kproj_h(2, 1), lambda: qproj_h(1, 0)],
                        6: [lambda: kproj_h(3, 1), lambda: qproj_h(1, 1)],
                        10: [lambda: tail_b(0, 0)],
                        13: [lambda: eb_load(1, 0)],
                        14: [lambda: eb_load(1, 1)],
                    }
                elif p == 0:
                    sched = {8: [lambda: tail_b(q4 - 1, 1)]}
                    for k in range(4):
                        sched[9 + 2 * k] = [
                            (lambda jj: lambda: oproj_full(q4 - 1, jj))(k)]
                else:
                    sched = {
                        10: [lambda: tail_b(q4, 0)],
                    }
                    if q4 < NQ4 - 1:
                        sched[4] = [lambda: qproj_h(q4 + 1, 0)]
                        sched[6] = [lambda: qproj_h(q4 + 1, 1)]
                        sched[13] = [lambda: eb_load(q4 + 1, 0)]
                        sched[14] = [lambda: eb_load(q4 + 1, 1)]
                for fn in sched.get(mt, ()):
                    fn()

            # ---- main pass loop -------------------------------------------
            for q4 in range(NQ4):
                for p in range(2):
                    state["y"] = pp.tile([65, 2, 512], F32, tag="y", bufs=1,
                                         name=f"y{q4}_{p}")
                    qk_round(q4, p, 0)
                    if q4 == 0 and p == 0:
                        vproj(0)
                    for mt in range(1, MT):
                        qk_round(q4, p, mt)
                        av_round(q4, p, mt - 1)
                        extra(q4, p, mt)
                    av_round(q4, p, MT - 1)
                    y_ps = state.pop("y")
                    nc.vector.tensor_copy(
                        y_sb[:, 2 * p:2 * p + 2, :], y_ps)
                    # denominators for this head pair: direct reciprocal on
                    # the single-partition accumulator row (no DMA reshape)
                    # reshape the denom row onto 32 partitions (32 DMA
                    # descriptors), cheap 32-lane reciprocal, DMA back to a
                    # row.  Consumers (tail_b) are scheduled 10+ rounds out
                    # so the descriptor latency is hidden.
                    d_t = work.tile([32, 32], F32R, tag="dt", bufs=2,
                                    name=f"dt{q4}_{p}")
                    nc.sync.dma_start(
                        out=d_t, in_=y_sb[64:65, 2 * p:2 * p + 2, :])
                    d_r = work.tile([32, 32], F32R, tag="dr", bufs=2,
                                    name=f"dr{q4}_{p}")
                    nc.vector.reciprocal(out=d_r.bitcast(F32),
                                         in_=d_t.bitcast(F32))
                    r_row = work.tile([1, 2, 512], F32R, tag="rrow", bufs=2,
                                      name=f"rr{q4}_{p}")
                    nc.sync.dma_start(out=r_row, in_=d_r)
                    state[("rrow", q4, p)] = r_row

            # final quarter tail: bridge with keep-warm matmuls, then the
            # last normalize + O-proj.
            warm2 = pp.tile([128, 512], F32, tag="s", bufs=2, name="warm2")
            for w in range(36):
                nc.tensor.matmul(warm2, identb_t, kT_all[:, 0, 0:512],
                                 start=(w == 0), stop=(w == 35))
            tail_b(NQ4 - 1, 1)
            for j in range(4):
                oproj_full(NQ4 - 1, j, tail=True)

    return nc


def _ensure_ntff_hook():
    """Register the axon NTFF profiling hook if the agent image lacks
    antenv.axon_hooks (profiling only; kernel runs fine without)."""
    try:
        from antenv.axon_hooks import get_axon_ntff_profile_hook  # noqa: F401
        return
    except ImportError:
        pass
    import types
    import antenv
    from trn_agent_boot.trn_boot import _ntff_profile_via_ctypes

    mod = types.ModuleType("antenv.axon_hooks")
    holder = {}
    mod.set_axon_ntff_profile_hook = lambda h: holder.__setitem__("h", h)
    mod.get_axon_ntff_profile_hook = lambda: holder.get("h")
    sys.modules["antenv.axon_hooks"] = mod
    antenv.axon_hooks = mod
    mod.set_axon_ntff_profile_hook(
        _ntff_profile_via_ctypes("/opt/axon/libaxon_pjrt.so"))


_NC_CACHE: dict = {}


def _get_nc() -> bass.Bass:
    if "nc" not in _NC_CACHE:
        _NC_CACHE["nc"] = _build_nc()
    return _NC_CACHE["nc"]


def _col_layout(v):
    """[256] per-core head-slice -> [128, 2] f32: row (h%2)*64+dk, col h//2."""
    return np.ascontiguousarray(
        v.reshape(2, 2, 64).transpose(1, 2, 0).reshape(128, 2)
    ).astype(np.float32)


def _w_hp_layout(w, bf):
    """W[:, dsl] (1024, 256) -> [128, 2(hp), 8(t), 128] bf16."""
    a = w.astype(bf).reshape(8, 128, 2, 128)     # [t, part, hp, col]
    return np.ascontiguousarray(a.transpose(1, 2, 0, 3))


def kernel(x, alpha, bias, Wq, bq, Wk, bk, Wv, bv, Wo, bo, trace=False):
    bf = ml_dtypes.bfloat16
    x = np.asarray(x, np.float32)
    alpha = np.asarray(alpha, np.float32)
    bias = np.asarray(bias, np.float32)
    Wq = np.asarray(Wq, np.float32); bq = np.asarray(bq, np.float32)
    Wk = np.asarray(Wk, np.float32); bk = np.asarray(bk, np.float32)
    Wv = np.asarray(Wv, np.float32); bv = np.asarray(bv, np.float32)
    Wo = np.asarray(Wo, np.float32); bo = np.asarray(bo, np.float32)

    c = np.ascontiguousarray

    in_maps = []
    per_b = {}
    for b in range(B):
        s = 1.0 + alpha[b]                             # (N,)
        xt = x[b].T.astype(bf)                         # (D, N)
        per_b[b] = {
            # SBUF layout [p, block, t, 512] -> contiguous 8KB block rows
            "xT": c(xt.reshape(8, 128, 4, 512).transpose(1, 2, 0, 3)),
            "ebiasT": c(np.exp(bias[b].T).astype(bf)),  # (N, N) [m, n]
            # (1+alpha) laid out per key partition: [p, mt] = s[mt*128+p]
            "scol": c(s.reshape(MT, 128).T.astype(np.float32)),
        }
    for core in range(NCORES):
        b, hg = divmod(core, 4)
        dsl = slice(hg * DSL, hg * DSL + DSL)
        in_maps.append({
            **per_b[b],
            "wq2": _w_hp_layout(Wq[:, dsl], bf),
            "wk2": _w_hp_layout(Wk[:, dsl], bf),
            "wv": c(Wv[:, dsl].astype(bf).reshape(8, 128, DSL).transpose(1, 0, 2)),
            "wo": c(Wo[dsl, :].astype(bf).reshape(2, 128, D).transpose(1, 0, 2)),
            "bv_r": c(bv[dsl].reshape(1, DSL).astype(bf)),
            "bq_col": _col_layout(0.125 * bq[dsl]),
            "bk_col": _col_layout(bk[dsl]),
            "ones64": np.ones((65, 64), np.float32),
            "onescol": np.ones((128, 1), bf),
            "onesr": np.ones((1, 128), bf),
            "identb": np.eye(128, dtype=bf),
        })

    if trace:
        _ensure_ntff_hook()
    nc = _get_nc()
    res = run_bass_kernel_spmd(
        nc, in_maps, core_ids=list(range(NCORES)), trace=trace)

    out = np.zeros((B, N, D), np.float32)
    for core in range(NCORES):
        out[core // 4] += res.results[core]["out_part"].astype(np.float32)
    out += bo[None, None, :]
    if trace:
        kernel.last_exec_time_ns = res.exec_time_ns
        kernel.last_profile = res.profile_json
    return out


# revision 30
# speedup vs baseline: 1.0038x; 1.0001x over previous
"""Biased self-attention TRN2 Bass kernel (8 NeuronCores), v6.

Problem: nn_BiasedSelfAttention — B=2, N=2048, D=1024, H=16, DK=64.
    q,k,v = split_heads(x@Wq+bq), ...; k,v scaled by (1+alpha[b,n]);
    logits = q k^T/sqrt(DK) + bias[b][None]; y = softmax(logits) v;
    out = merge_heads(y) @ Wo + bo.

Sharding: 8 cores = (batch b in {0,1}) x (head-group hg in {0..3} of 4
heads = 256 dims of D).  Data parallel over B, tensor parallel over H.
Each core computes a partial O-projection (its 256 rows of Wo); the
host sums the 4 partials per batch and adds bo.

v6 design (v5 profile: rounds fully ACT-bound at 1325ns because the
FD=512 exp pays ~260ns/instr overhead twice; tail_b at round 4 of each
quarter blocked the PE FIFO ~7us on the reciprocal's SBUF->SBUF DMA
chain; PSUM had no spare banks to decouple projections from QK):
  - quarters split into two 16-round PASSES (head pair = pass).  PSUM:
    qk s-tag [128,2,512] x2 (4 banks) + y [65,2,512] (2 banks) + small
    s-tag x2 (2 banks) = 8.  Projections/O-proj/tail use the small tag
    so they never steal the QK double-buffer.
  - exp is ONE FD=1024 ACTIVATE per round again (PSUM source, per-key
    (1+alpha) scale AP); DVE bf16 2x multiply by host-precomputed
    exp(bias); ebias tiles are loaded once per quarter and reused by
    both passes.
  - denominators: per-pass DVE reciprocal directly on the single-
    partition accumulator row -- the DMA reshape chain is gone.
  - k/q weights are loaded in hp halves so round 0 starts ~21us in.
"""

import json
import sys

sys.path.insert(0, "/opt/trn_rl_repo")

import numpy as np
import ml_dtypes

import concourse.bass as bass
import concourse.mybir as mybir
import concourse.tile as tile
from concourse.bass_utils import run_bass_kernel_spmd

# ---------------------------------------------------------------- bir fix --
# The pinned walrus encodes at most ONE sem-wait per instruction, but Tile's
# wait-assigner can emit several.  Hoist extras onto EventSemaphore
# instructions just before the instruction.


def _split_multi_waits(bir_json: bytes) -> bytes:
    m = json.loads(bir_json)
    for fn in m.get("functions", []):
        for blk in fn.get("blocks", []):
            insts = blk.get("instructions")
            if not insts:
                continue
            out = []
            for inst in insts:
                sync = inst.get("sync_info")
                waits = (sync or {}).get("on_wait") or []
                if len(waits) > 1:
                    for i, w in enumerate(waits[:-1]):
                        out.append({
                            "debug": inst.get("debug", 0),
                            "engine": inst["engine"],
                            "ins": [],
                            "name": f"{inst['name']}-sw{i}",
                            "opcode": "EventSemaphore",
                            "outs": [],
                            "sync_info": {"on_update": [], "on_wait": [w]},
                        })
                    sync["on_wait"] = waits[-1:]
                out.append(inst)
            blk["instructions"] = out
    return json.dumps(m).encode()


def _patch_bass():
    if getattr(bass.Bass, "_multiwait_patched", False):
        return
    orig = bass.Bass.to_json_bytes

    def to_json_bytes(self, *a, **kw):
        return _split_multi_waits(orig(self, *a, **kw))

    bass.Bass.to_json_bytes = to_json_bytes
    bass.Bass._multiwait_patched = True


_patch_bass()


def _patch_ldw_opt():
    """Enable walrus's redundant-LDWEIGHTS elimination (off by default in
    bass_utils).  Consecutive matmuls that share a stationary operand then
    load it once."""
    import concourse.bass_utils as _bu
    if getattr(_bu, "_ldw_opt_patched", False):
        return
    orig = _bu.run_command

    def run_command(cmd, *a, **kw):
        # ldw-opt=true crashes walrus codegen (visitInstLdweights) on this
        # pinned compiler -- keep the flag off; wrapper retained as a hook.
        return orig(cmd, *a, **kw)

    _bu.run_command = run_command
    _bu._ldw_opt_patched = True


_patch_ldw_opt()

# ------------------------------------------------------------- dimensions --
B, N, D, H = 2, 2048, 1024, 16
DK = D // H                      # 64
NCORES = 8
HPC = H // 4                     # 4 heads per core
DSL = HPC * DK                   # 256 D-columns per core
NQ4 = N // 512                   # 4 query quarters
MT = N // 128                    # 16 key tiles
F32 = mybir.dt.float32
F32R = mybir.dt.float32r
BF16 = mybir.dt.bfloat16
Exp = mybir.ActivationFunctionType.Exp
Copy = mybir.ActivationFunctionType.Copy
Add = mybir.AluOpType.add
Mult = mybir.AluOpType.mult


def _build_nc() -> bass.Bass:
    nc = bass.Bass()

    xT = nc.dram_tensor("xT", [128, 4, 8, 512], BF16, kind="ExternalInput")
    wq2 = nc.dram_tensor("wq2", [128, 2, 8, 128], BF16, kind="ExternalInput")
    wk2 = nc.dram_tensor("wk2", [128, 2, 8, 128], BF16, kind="ExternalInput")
    wv = nc.dram_tensor("wv", [128, 8, DSL], BF16, kind="ExternalInput")
    wo = nc.dram_tensor("wo", [128, 2, D], BF16, kind="ExternalInput")
    ebiasT = nc.dram_tensor("ebiasT", [N, N], BF16, kind="ExternalInput")
    bv_r = nc.dram_tensor("bv_r", [1, DSL], BF16, kind="ExternalInput")
    bq_col = nc.dram_tensor("bq_col", [128, 2], F32, kind="ExternalInput")
    bk_col = nc.dram_tensor("bk_col", [128, 2], F32, kind="ExternalInput")
    scol = nc.dram_tensor("scol", [128, MT], F32, kind="ExternalInput")
    ones64 = nc.dram_tensor("ones64", [65, 64], F32R, kind="ExternalInput")
    onescol = nc.dram_tensor("onescol", [128, 1], BF16, kind="ExternalInput")
    onesr = nc.dram_tensor("onesr", [1, 128], BF16, kind="ExternalInput")
    identb = nc.dram_tensor("identb", [128, 128], BF16, kind="ExternalInput")
    out_part = nc.dram_tensor("out_part", [N, D], BF16, kind="ExternalOutput")

    with tile.TileContext(nc) as tc:
        with tc.tile_pool(name="consts", bufs=1) as consts, \
             tc.tile_pool(name="persist", bufs=1) as persist, \
             tc.tile_pool(name="stream", bufs=4) as stream, \
             tc.tile_pool(name="work", bufs=3) as work, \
             tc.tile_pool(name="outp", bufs=2) as outp, \
             tc.tile_pool(name="psum", bufs=1, space="PSUM") as pp:

            # ---- constants -------------------------------------------------
            xT_sb = consts.tile([128, 4, 8, 512], BF16, tag="xT")
            wq_t = consts.tile([128, 2, 8, 128], BF16, tag="wq")
            wk_t = consts.tile([128, 2, 8, 128], BF16, tag="wk")
            wv_t = consts.tile([128, 8, DSL], BF16, tag="wv")
            wo_t = consts.tile([128, 2, D], BF16, tag="wo")
            identb_t = consts.tile([128, 128], BF16, tag="identb")
            bv_t = consts.tile([1, DSL], BF16, tag="bv")
            bq_c = consts.tile([128, 2], F32, tag="bqc")
            bk_c = consts.tile([128, 2], F32, tag="bkc")
            scol_t = consts.tile([128, MT], F32, tag="scol")
            ones64_t = consts.tile([65, 64], F32R, tag="ones64")
            onescol_t = consts.tile([128, 1], BF16, tag="onescol")
            onesr_t = consts.tile([1, 128], BF16, tag="onesr")
            # DMA order = arrival order (~175 GB/s effective, ~9us startup).
            nc.sync.dma_start(out=identb_t, in_=identb[:])
            nc.sync.dma_start(out=onescol_t, in_=onescol[:])
            nc.sync.dma_start(out=onesr_t, in_=onesr[:])
            nc.sync.dma_start(out=bq_c, in_=bq_col[:])
            nc.sync.dma_start(out=bk_c, in_=bk_col[:])
            nc.sync.dma_start(out=scol_t, in_=scol[:])
            nc.sync.dma_start(out=xT_sb[:, 0], in_=xT[:, 0])
            nc.sync.dma_start(out=wk_t[:, 0], in_=wk2[:, 0])
            nc.sync.dma_start(out=wq_t[:, 0], in_=wq2[:, 0])
            nc.sync.dma_start(out=wv_t, in_=wv[:])
            nc.sync.dma_start(out=bv_t, in_=bv_r[:])
            nc.sync.dma_start(out=ones64_t, in_=ones64[:])

            # ---- persistent intermediates ---------------------------------
            # q^T/k^T: [dk-pair row hi*64+dk, head-pair hp, n]; kT UNSCALED
            qT_all = persist.tile([128, 2, N], BF16, tag="qT")
            kT_all = persist.tile([128, 2, N], BF16, tag="kT")
            # v (scaled) + ones col: [m-part, m-tile, head, 65]
            vaug = persist.tile([128, MT, HPC, 65], BF16, tag="vaug")
            # normalized y^T for O-proj
            yT_all = persist.tile([128, 2, N], BF16, tag="yT")
            # per-quarter y + denominators staging
            y_sb = persist.tile([65, HPC, 512], F32R, tag="ysb")

            # vaug ones columns, written once
            nc.vector.tensor_copy(
                vaug[:, :, :, 64:65],
                onescol_t.unsqueeze(1).unsqueeze(1)
                .broadcast_to([128, MT, HPC, 1]))

            state = {}

            def eb_load(q4, mt):
                eb_t = stream.tile([128, 512], BF16, tag="ebias", bufs=18,
                                   name=f"b{q4}_{mt}")
                nc.sync.dma_start(
                    out=eb_t,
                    in_=ebiasT[mt * 128:mt * 128 + 128,
                               q4 * 512:q4 * 512 + 512])
                state[("eb", q4, mt)] = eb_t

            # prefetch ALL q0 bias tiles, interleaved with the remaining
            # x blocks in exact deadline order: kproj_h(1,0) at round ~2
            # needs xT1 almost immediately, so only two bias tiles go
            # ahead of it; later blocks have progressively more slack.
            nc.sync.dma_start(out=xT_sb[:, 1], in_=xT[:, 1])
            eb_load(0, 0)
            eb_load(0, 1)
            eb_load(0, 2)
            eb_load(0, 3)
            nc.sync.dma_start(out=xT_sb[:, 2], in_=xT[:, 2])
            for mt in range(4, 10):
                eb_load(0, mt)
            nc.sync.dma_start(out=xT_sb[:, 3], in_=xT[:, 3])
            for mt in range(10, 16):
                eb_load(0, mt)
            nc.sync.dma_start(out=wk_t[:, 1], in_=wk2[:, 1])
            nc.sync.dma_start(out=wq_t[:, 1], in_=wq2[:, 1])
            nc.sync.dma_start(out=wo_t, in_=wo[:])

            # ---- projections (small s-tag PSUM, interleaved into rounds) --
            def kproj_h(c, hp):
                nsl = slice(c * 512, c * 512 + 512)
                ps = pp.tile([128, 512], F32, tag="s", bufs=2,
                             name=f"kps{c}_{hp}")
                for t in range(8):
                    nc.tensor.matmul(
                        ps, wk_t[:, hp, t, :], xT_sb[:, c, t, :],
                        start=(t == 0), stop=(t == 7))
                nc.vector.tensor_scalar(
                    kT_all[:, hp, nsl], ps, bk_c[:, hp:hp + 1], None, Add)

            def qproj_h(c, hp):
                nsl = slice(c * 512, c * 512 + 512)
                ps = pp.tile([128, 512], F32, tag="s", bufs=2,
                             name=f"qps{c}_{hp}")
                for t in range(8):
                    nc.tensor.matmul(
                        ps, wq_t[:, hp, t, :], xT_sb[:, c, t, :],
                        start=(t == 0), stop=(t == 7))
                nc.vector.tensor_scalar(
                    qT_all[:, hp, nsl], ps,
                    0.125, bq_c[:, hp:hp + 1], Mult, Add)

            def vproj(mt):
                mb, mo = divmod(mt, 4)
                vp = pp.tile([128, 256], F32, tag="s", bufs=2,
                             name=f"vps{mt}")
                for t in range(8):
                    nc.tensor.matmul(
                        vp, xT_sb[:, mb, t, mo * 128:mo * 128 + 128],
                        wv_t[:, t, :], start=(t == 0), stop=False)
                nc.tensor.matmul(
                    vp, onesr_t[0:1, :], bv_t[0:1, :], start=False, stop=True)
                vr = vp.rearrange("p (h d) -> p h d", h=HPC)
                nc.vector.tensor_scalar(
                    vaug[:, mt, :, 0:64], vr,
                    scol_t[:, mt:mt + 1], None, Mult)

            warm = pp.tile([128, 512], F32, tag="s", bufs=2, name="warm")
            for w in range(52):
                nc.tensor.matmul(warm[:, 0:128], identb_t, identb_t,
                                 start=(w == 0), stop=(w == 51))

            # minimum to start round 0 of pass (0,0): kT/qT chunk 0 only.
            # vproj(0..) rides the round inserts -- AV can lag a few rounds
            # behind the exp cadence (a/e bufs below give the headroom).
            kproj_h(0, 0)
            qproj_h(0, 0)

            # ---- round bodies ---------------------------------------------
            def qk_round(q4, p, mt):
                nsl = slice(q4 * 512, q4 * 512 + 512)
                if p == 0 and ("eb", q4, mt) not in state:
                    eb_load(q4, mt)
                eb_t = state[("eb", q4, mt)]
                if p == 1:
                    del state[("eb", q4, mt)]
                s_ps = pp.tile([128, 2, 512], F32, tag="s2", bufs=2,
                               name=f"s{q4}_{p}_{mt}")
                for hi in range(2):
                    nc.tensor.matmul(
                        s_ps[:, hi],
                        kT_all[hi * 64:hi * 64 + 64, p,
                               mt * 128:mt * 128 + 128],
                        qT_all[hi * 64:hi * 64 + 64, p, nsl],
                        start=True, stop=True)
                e_t = work.tile([128, 2, 512], BF16, tag="e", bufs=8,
                                name=f"e{q4}_{p}_{mt}")
                # per-key-partition (1+alpha) scale rides the exp
                nc.scalar.activation(e_t, s_ps, Exp,
                                     scale=scol_t[:, mt:mt + 1])
                a_t = work.tile([128, 2, 512], BF16, tag="a", bufs=8,
                                name=f"a{q4}_{p}_{mt}")
                nc.vector.tensor_mul(
                    a_t, e_t,
                    eb_t.unsqueeze(1).broadcast_to([128, 2, 512]))
                state[("a", mt % 8)] = a_t

            def av_round(q4, p, mt):
                a_t = state[("a", mt % 8)]
                y_ps = state["y"]
                for hi in range(2):
                    nc.tensor.matmul(
                        y_ps[:, hi], vaug[:, mt, 2 * p + hi, :], a_t[:, hi],
                        start=(mt == 0), stop=(mt == MT - 1))

            def tail_b(q4, hq):
                # one head-pair: 2 recip-broadcast matmuls (PE) + 2 muls (DVE)
                r_row = state[("rrow", q4, hq)]
                for hi in range(2):
                    h = hq * 2 + hi
                    rb = pp.tile([128, 512], F32, tag="s", bufs=2,
                                 name=f"rb{q4}_{h}")
                    nc.tensor.matmul(
                        rb[0:64, :], ones64_t[0:1, :],
                        r_row[0:1, hi, :], start=True, stop=True)
                    nc.vector.tensor_mul(
                        yT_all[hi * 64:hi * 64 + 64, hq,
                               q4 * 512:q4 * 512 + 512],
                        y_sb[0:64, h, :].bitcast(F32), rb[0:64, :])

            def oproj_full(q4, j, tail=False):
                # both dc halves together: the yT stationary operand is
                # shared by consecutive matmuls, so with ldw-opt walrus
                # loads it once per hp instead of once per matmul.
                nt = q4 * 4 + j
                o_ps = [pp.tile([128, 512], F32, tag="s", bufs=2,
                                name=f"o{nt}_{dc}") for dc in range(2)]
                for hp in range(2):
                    for dc in range(2):
                        nc.tensor.matmul(
                            o_ps[dc],
                            yT_all[:, hp, nt * 128:nt * 128 + 128],
                            wo_t[:, hp, dc * 512:dc * 512 + 512],
                            start=(hp == 0), stop=(hp == 1))
                for dc in range(2):
                    o_sb = outp.tile([128, 512], BF16, tag="osb",
                                     name=f"ob{nt}_{dc}")
                    # in the final tail ACT is idle: alternate copy engines
                    if tail and dc == 1:
                        nc.scalar.copy(o_sb, o_ps[dc])
                    else:
                        nc.vector.tensor_copy(o_sb, o_ps[dc])
                    # gpsimd queue: keeps compute-gated stores from head-of-
                    # line blocking the ebias loads on the sync queue
                    (nc.sync if tail and dc == 1 else nc.gpsimd).dma_start(
                        out=out_part[nt * 128:nt * 128 + 128,
                                     dc * 512:dc * 512 + 512], in_=o_sb)

            # insertion schedules: {(q4==0, p): {mt: [fns]}} built inline
            def extra(q4, p, mt):
                if q4 == 0 and p == 0:
                    sched = {
                        1: [lambda: vproj(1)],
                        2: [lambda: kproj_h(1, 0), lambda: vproj(2)],
                        3: [lambda: vproj(3)], 4: [lambda: vproj(4)],
                        5: [lambda: vproj(5)],
                        6: [lambda: kproj_h(2, 0), lambda: vproj(6)],
                        7: [lambda: vproj(7)], 8: [lambda: vproj(8)],
                        9: [lambda: vproj(9)],
                        10: [lambda: kproj_h(3, 0), lambda: vproj(10)],
                        11: [lambda: vproj(11)], 12: [lambda: vproj(12)],
                        13: [lambda: vproj(13)],
                        14: [lambda: vproj(14), lambda: qproj_h(0, 1)],
                        15: [lambda: vproj(15), lambda: kproj_h(0, 1)],
                    }
                elif q4 == 0 and p == 1:
                    sched = {
                        2: [lambda: kproj_h(1, 1)],
                        4: [lambda: kproj_h(2, 1), lambda: qproj_h(1, 0)],
                        6: [lambda: kproj_h(3, 1), lambda: qproj_h(1, 1)],
                        10: [lambda: tail_b(0, 0)],
                        13: [lambda: eb_load(1, 0)],
                        14: [lambda: eb_load(1, 1)],
                    }
                elif p == 0:
                    sched = {8: [lambda: tail_b(q4 - 1, 1)]}
                    for k in range(4):
                        sched[9 + 2 * k] = [
                            (lambda jj: lambda: oproj_full(q4 - 1, jj))(k)]
                else:
                    sched = {
                        10: [lambda: tail_b(q4, 0)],
                    }
                    if q4 < NQ4 - 1:
                        sched[4] = [lambda: qproj_h(q4 + 1, 0)]
                        sched[6] = [lambda: qproj_h(q4 + 1, 1)]
                        sched[13] = [lambda: eb_load(q4 + 1, 0)]
                        sched[14] = [lambda: eb_load(q4 + 1, 1)]
                for fn in sched.get(mt, ()):
                    fn()

            # ---- main pass loop -------------------------------------------
            for q4 in range(NQ4):
                for p in range(2):
                    state["y"] = pp.tile([65, 2, 512], F32, tag="y", bufs=1,
                                         name=f"y{q4}_{p}")
                    qk_round(q4, p, 0)
                    if q4 == 0 and p == 0:
                        vproj(0)
                    for mt in range(1, MT):
                        qk_round(q4, p, mt)
                        av_round(q4, p, mt - 1)
                        extra(q4, p, mt)
                    av_round(q4, p, MT - 1)
                    y_ps = state.pop("y")
                    nc.vector.tensor_copy(
                        y_sb[:, 2 * p:2 * p + 2, :], y_ps)
                    # denominators for this head pair: direct reciprocal on
                    # the single-partition accumulator row (no DMA reshape)
                    # reshape the denom row onto 32 partitions (32 DMA
                    # descriptors), cheap 32-lane reciprocal, DMA back to a
                    # row.  Consumers (tail_b) are scheduled 10+ rounds out
                    # so the descriptor latency is hidden.
                    d_t = work.tile([32, 32], F32R, tag="dt", bufs=2,
                                    name=f"dt{q4}_{p}")
                    nc.sync.dma_start(
                        out=d_t, in_=y_sb[64:65, 2 * p:2 * p + 2, :])
                    d_r = work.tile([32, 32], F32R, tag="dr", bufs=2,
                                    name=f"dr{q4}_{p}")
                    nc.vector.reciprocal(out=d_r.bitcast(F32),
                                         in_=d_t.bitcast(F32))
                    r_row = work.tile([1, 2, 512], F32R, tag="rrow", bufs=2,
                                      name=f"rr{q4}_{p}")
                    nc.sync.dma_start(out=r_row, in_=d_r)
                    state[("rrow", q4, p)] = r_row

            # final quarter tail: bridge with keep-warm matmuls, then the
            # last normalize + O-proj.
            warm2 = pp.tile([128, 512], F32, tag="s", bufs=2, name="warm2")
            for w in range(36):
                nc.tensor.matmul(warm2, identb_t, kT_all[:, 0, 0:512],
                                 start=(w == 0), stop=(w == 35))
            tail_b(NQ4 - 1, 1)
            for j in range(4):
                oproj_full(NQ4 - 1, j, tail=True)

    return nc


def _ensure_ntff_hook():
    """Register the axon NTFF profiling hook if the agent image lacks
    antenv.axon_hooks (profiling only; kernel runs fine without)."""
    try:
        from antenv.axon_hooks import get_axon_ntff_profile_hook  # noqa: F401
        return
    except ImportError:
        pass
    import types
    import antenv
    from trn_agent_boot.trn_boot import _ntff_profile_via_ctypes

    mod = types.ModuleType("antenv.axon_hooks")
    holder = {}
    mod.set_axon_ntff_profile_hook = lambda h: holder.__setitem__("h", h)
    mod.get_axon_ntff_profile_hook = lambda: holder.get("h")
    sys.modules["antenv.axon_hooks"] = mod
    antenv.axon_hooks = mod
    mod.set_axon_ntff_profile_hook(
        _ntff_profile_via_ctypes("/opt/axon/libaxon_pjrt.so"))


_NC_CACHE: dict = {}


def _get_nc() -> bass.Bass:
    if "nc" not in _NC_CACHE:
        _NC_CACHE["nc"] = _build_nc()
    return _NC_CACHE["nc"]


def _col_layout(v):
    """[256] per-core head-slice -> [128, 2] f32: row (h%2)*64+dk, col h//2."""
    return np.ascontiguousarray(
        v.reshape(2, 2, 64).transpose(1, 2, 0).reshape(128, 2)
    ).astype(np.float32)


def _w_hp_layout(w, bf):
    """W[:, dsl] (1024, 256) -> [128, 2(hp), 8(t), 128] bf16."""
    a = w.astype(bf).reshape(8, 128, 2, 128)     # [t, part, hp, col]
    return np.ascontiguousarray(a.transpose(1, 2, 0, 3))


def kernel(x, alpha, bias, Wq, bq, Wk, bk, Wv, bv, Wo, bo, trace=False):
    bf = ml_dtypes.bfloat16
    x = np.asarray(x, np.float32)
    alpha = np.asarray(alpha, np.float32)
    bias = np.asarray(bias, np.float32)
    Wq = np.asarray(Wq, np.float32); bq = np.asarray(bq, np.float32)
    Wk = np.asarray(Wk, np.float32); bk = np.asarray(bk, np.float32)
    Wv = np.asarray(Wv, np.float32); bv = np.asarray(bv, np.float32)
    Wo = np.asarray(Wo, np.float32); bo = np.asarray(bo, np.float32)

    c = np.ascontiguousarray

    in_maps = []
    per_b = {}
    for b in range(B):
        s = 1.0 + alpha[b]                             # (N,)
        xt = x[b].T.astype(bf)                         # (D, N)
        per_b[b] = {
            # SBUF layout [p, block, t, 512] -> contiguous 8KB block rows
            "xT": c(xt.reshape(8, 128, 4, 512).transpose(1, 2, 0, 3)),
            "ebiasT": c(np.exp(bias[b].T).astype(bf)),  # (N, N) [m, n]
            # (1+alpha) laid out per key partition: [p, mt] = s[mt*128+p]
            "scol": c(s.reshape(MT, 128).T.astype(np.float32)),
        }
    for core in range(NCORES):
        b, hg = divmod(core, 4)
        dsl = slice(hg * DSL, hg * DSL + DSL)
        in_maps.append({
            **per_b[b],
            "wq2": _w_hp_layout(Wq[:, dsl], bf),
            "wk2": _w_hp_layout(Wk[:, dsl], bf),
            "wv": c(Wv[:, dsl].astype(bf).reshape(8, 128, DSL).transpose(1, 0, 2)),
            "wo": c(Wo[dsl, :].astype(bf).reshape(2, 128, D).transpose(1, 0, 2)),
            "bv_r": c(bv[dsl].reshape(1, DSL).astype(bf)),
            "bq_col": _col_layout(0.125 * bq[dsl]),
            "bk_col": _col_layout(bk[dsl]),
            "ones64": np.ones((65, 64), np.float32),
            "onescol": np.ones((128, 1), bf),
            "onesr": np.ones((1, 128), bf),
            "identb": np.eye(128, dtype=bf),
        })

    if trace:
        _ensure_ntff_hook()
    nc = _get_nc()
    res = run_bass_kernel_spmd(
        nc, in_maps, core_ids=list(range(NCORES)), trace=trace)

    out = np.zeros((B, N, D), np.float32)
    for core in range(NCORES):
        out[core // 4] += res.results[core]["out_part"].astype(np.float32)
    out += bo[None, None, :]
    if trace:
        kernel.last_exec_time_ns = res.exec_time_ns
        kernel.last_profile = res.profile_json
    return out
